# revision 28
# baseline (speedup 1.0000x reference)
"""Trainium2 Bass kernel for nn_NeuronS3DiffUpsample2D.

Reference computation (per sample b):
    up   = nearest-2x-upsample(x[b])                       # [C, 320, 320]
    w    = Wb + 0.25 * einsum('or,rikl->oikl', lora_up, lora_down)
    w_b  = w * de_mod[b, None, :, None, None]              # modulate input chans
    dem  = rsqrt(sum_{i,k,l} w_b^2 + eps)                  # per output chan
    y[b] = conv2d(up, w_b * dem, SAME) + bias

Key algebraic transform: a 3x3 SAME conv on a 2x nearest-upsampled image
decomposes into 4 output phases (di, dj in {0,1}), each a 2x2 conv on the
ORIGINAL 160x160 input:
    y[2i+di, 2j+dj] = sum_{a,b in {0,1}} K[di,dj,a,b] @ x[i+a+di-1, j+b+dj-1]
where the 16 [O, I] matrices K are sums of 1/2/4 of the 9 taps of w.
This is 4/9 of the naive FLOPs and never materializes the upsampled image.

On top of that, the steady-state loop uses a 14-matmul block: the two di=0
phases share three column-tap tiles t[kj][i,j] = sum_a w[S(0,a),kj] (x) at
row shifts (6 matmuls, PE) and are assembled by the DVE as
    y[2i, 2j+dj] = t0[j+dj] + t1[j+1] + t2[j+1+dj] + bias
(shifted overlapping-pair access patterns; t1 uses a 0-stride broadcast
pair).  The di=1 phases stay as direct 4-tap accumulations (8 matmuls)
evicted by the ACT engine.  This trades 2 of 16 PE matmuls per block for
DVE/ACT work that fits in their idle capacity: PE is the only saturated
engine (~90% busy at 1 col/cycle).

For the assembly to be a pure add, the demod scale is folded INTO the
weights.  demod depends on the weights themselves, so the first KSWITCH
blocks run the original 16-matmul path (demod applied per-partition at PSUM
eviction) while the fold chain (row-form demod via a [1,C] PE reduction, a
broadcast outer product, and a DVE multiply) completes in the background.

Sharding: data-parallel over batch B=8 across 8 NeuronCores; each core builds
its own per-sample weights locally.  Host-side work is layout only (slicing,
transposition, fp32->bf16 rounding).  All arithmetic is on device.

Everything runs in bf16 (PE streams bf16 at 1 cycle/row like f32r, but
LDWEIGHTS fully hides under the previous 480-col stream, and input/output
DMA halves; measured rel err 4e-3 vs the 2e-2 budget).  Output goes to DRAM
as bf16 and is widened to f32 on the host.

The input is banded with a short first band so the first conv matmul isn't
gated behind a 1.2MB transfer; weight DMAs ride the ACT queue, dmbias the
sync queue, bands the gpsimd SWDGE queue.
"""

import sys
import numpy as np
import ml_dtypes
from contextlib import ExitStack

try:
    import concourse.bass as bass
except ImportError:  # grading env without the axon PYTHONPATH
    sys.path.insert(0, "/opt/trn_rl_repo")
    import concourse.bass as bass
import concourse.tile as tile
from concourse import bacc, mybir
from concourse.bass_utils import run_bass_kernel_spmd

B, C, H, W = 8, 128, 160, 160
RANK = 32
SCALING = 0.25
EPS = 1e-8
WP = W + 2          # padded row length (zero col on each side)
R_BLK = 3           # x-rows per matmul block -> N = 3*160 = 480 <= 512
# band cut points: short first band so block 0 starts early
CUTS = [0, 15, 42, 69, 96, 123, 150, 160]
KSWITCH = 5         # blocks on the 16-matmul path while demod-fold completes
NCORES = 8

f32 = mybir.dt.float32
bf16 = mybir.dt.bfloat16


def _conv_kernel(ctx, tc, y, x, dmbias, wbT, luT, ldT, ident2):
    nc = tc.nc
    AF = mybir.ActivationFunctionType
    ALU = mybir.AluOpType
    AX = mybir.AxisListType

    const = ctx.enter_context(tc.tile_pool(name="const", bufs=1))
    bands = ctx.enter_context(tc.tile_pool(name="bands", bufs=7))

    # dmbias/ident2 (tiny, gate the de_mod transpose) on sync; weight
    # tensors on the otherwise-idle ACT queue; bands on gpsimd SWDGE.
    dmbR = const.tile([2, C], f32)
    nc.sync.dma_start(dmbR[:], dmbias[:])
    id2 = const.tile([2, 2], f32)
    nc.sync.dma_start(id2[:], ident2[:])
    # WbT split across the sync and ACT queues so the two halves transfer
    # in parallel (a single 295KB DMA gated the whole weight chain)
    WbTS = const.tile([128, 9, C], bf16)         # Wb^T: [i, t, o]
    nc.sync.dma_start(WbTS[:, 0:5, :], wbT[:, 0 : 5 * C])
    LD9 = const.tile([RANK, 9, C], bf16)         # lora_down^T: [r, t, i]
    nc.scalar.dma_start(LD9[:], ldT[:])
    LUTn = const.tile([RANK, C], bf16)           # lora_up^T: [r, o]
    nc.scalar.dma_start(LUTn[:], luT[:])
    nc.scalar.dma_start(WbTS[:, 5:9, :], wbT[:, 5 * C : 9 * C])

    # weight tensors the conv loop reads as stationary operands
    wm3 = const.tile([128, 9, C], bf16)          # modulated w^T: [i, t, o]
    R01 = const.tile([128, 3, C], bf16)          # rows ki1+ki2
    R10 = const.tile([128, 3, C], bf16)          # rows ki0+ki1
    comb8 = const.tile([128, 8, C], bf16)        # two-column tap sums (unfolded)
    wm3F = const.tile([128, 9, C], bf16)         # demod-folded variants
    R01F = const.tile([128, 3, C], bf16)
    R10F = const.tile([128, 3, C], bf16)
    comb4F = const.tile([128, 4, C], bf16)       # folded di=1 two-column sums
    demP = const.tile([128, 1], f32)             # rsqrt demod, per output chan
    dmb = const.tile([128, 3], f32)              # de_mod[i], bias[o], 0.25*de_mod
    s2 = const.tile([128, C], f32)               # per-(i,o) tap-summed squares
    onesS = const.tile([128, 1], f32)
    onesRow = const.tile([1, C], f32)

    nc.vector.memset(onesS[:], 1.0)
    nc.vector.memset(onesRow[:], 1.0)

    # ---- input bands: (lo, hi) are halo-inclusive x-row bounds.  x arrives
    # pre-padded on the host ([C, H+2, W+2] with zero borders), so a single
    # whole-tile contiguous DMA delivers data AND halos: no on-device border
    # writes (a disjoint border zero-write races with the DMA's write
    # granules on HW), and per-partition descriptors coalesce.
    segs = [(CUTS[i] - 1, min(CUTS[i + 1], H)) for i in range(len(CUTS) - 1)]
    # DMA issue order: bands 0 and 1 first (consumed earliest), then the
    # LAST band (the tail block is processed early, see below), then the
    # rest in order.
    dma_order = [0, 1, len(segs) - 1] + list(range(2, len(segs) - 1))
    band_tiles = [None] * len(segs)
    for si in dma_order:
        lo, hi = segs[si]
        nrows = hi - lo + 1
        bt = bands.tile([128, nrows, WP], bf16, tag="band", name=f"band{lo}")
        nc.gpsimd.dma_start(bt[:], x[:, lo + 1 : hi + 2, :])
        band_tiles[si] = (bt, lo, hi)

    def _band_for(i0, R):
        for bt, lo, hi in band_tiles:
            if lo <= i0 - 1 and i0 + R <= hi:
                return bt, lo
        raise AssertionError(f"no band for block {i0}")

    # ---- weight stage ----
    with tc.tile_pool(name="wtmp", bufs=1) as wtmp, tc.tile_pool(
        name="wpsum", bufs=1, space="PSUM"
    ) as wpsum:
        dmbP = wpsum.tile([128, 2], f32)
        nc.tensor.transpose(dmbP[:], dmbR[:], id2[:])
        nc.vector.tensor_copy(dmb[:, 0:2], dmbP[:])
        nc.vector.tensor_scalar_mul(dmb[:, 2:3], dmb[:, 0:1], SCALING)

        # deltaT_unscaled[i, t, o] = sum_r down[r,i,t] * up[o,r]; the 0.25
        # lora scale rides in via the fused modulation below
        deltaP = wpsum.tile([128, 9, C], f32)
        for t in range(9):
            nc.tensor.matmul(
                deltaP[:, t, :], LD9[:, t, :], LUTn[:], start=True, stop=True
            )

        # wm3 = Wb^T*dm + deltaT*(0.25*dm)
        WbTm = wtmp.tile([128, 9, C], bf16)
        nc.vector.tensor_scalar_mul(WbTm[:], WbTS[:], dmb[:, 0:1])
        nc.vector.scalar_tensor_tensor(
            wm3[:], deltaP[:], dmb[:, 2:3], WbTm[:],
            op0=ALU.mult, op1=ALU.add,
        )

        # Row combos over ki (t = 3*ki + kj):
        #   (di=0, a=0): ki0        (di=0, a=1): ki1+ki2
        #   (di=1, a=0): ki0+ki1    (di=1, a=1): ki2
        # Single-column taps are read directly out of wm3/R01/R10; only the
        # two-column sums are materialized, phase-0 slots first.
        nc.vector.tensor_add(R01[:], wm3[:, 3:6, :], wm3[:, 6:9, :])
        nc.vector.tensor_add(R10[:], wm3[:, 0:3, :], wm3[:, 3:6, :])
        rowsrc = {
            (0, 0): wm3[:, 0:3, :],
            (0, 1): R01[:],
            (1, 0): R10[:],
            (1, 1): wm3[:, 6:9, :],
        }
        for p in range(4):
            di, dj = p >> 1, p & 1
            for a in range(2):
                S = rowsrc[(di, a)]
                dst = comb8[:, 4 * di + 2 * a + dj, :]
                if dj == 0:      # (dj=0, b=1): kj1 + kj2
                    nc.vector.tensor_add(dst, S[:, 1, :], S[:, 2, :])
                else:            # (dj=1, b=0): kj0 + kj1
                    nc.vector.tensor_add(dst, S[:, 0, :], S[:, 1, :])

        # demod source: sq3 = wm3^2 (ACT), tap-sum on DVE.  The partition
        # sums (PE matmuls) are emitted inside the conv loop so the in-order
        # tensor queue doesn't stall the conv behind this reduce.
        sq3 = wtmp.tile([128, 9, C], f32)
        nc.scalar.square(sq3[:], wm3[:])
        nc.vector.tensor_reduce(
            s2[:], sq3.rearrange("p t o -> p o t"), axis=AX.X, op=ALU.add
        )

    def _conv_weight(di, dj, a, b):
        if dj == 0 and b == 0:
            return rowsrc[(di, a)][:, 0, :]
        if dj == 1 and b == 1:
            return rowsrc[(di, a)][:, 2, :]
        return comb8[:, 4 * di + 2 * a + dj, :]

    rowsrcF = {(1, 0): R10F[:], (1, 1): wm3F[:, 6:9, :]}

    def _conv_weightF(dj, a, b):  # di=1 only
        if dj == 0 and b == 0:
            return rowsrcF[(1, a)][:, 0, :]
        if dj == 1 and b == 1:
            return rowsrcF[(1, a)][:, 2, :]
        return comb4F[:, 2 * a + dj, :]

    def _pair_view(tt, col0):
        """[128, R, W, 2] view of a [128, R, WP] tile: (j, d) -> col j+d+col0."""
        ap = tt[:].copy()
        ap.ap = ap.ap[:-1] + [[1, W], [1, 2]]
        ap.offset = ap.offset + col0
        return ap

    # ---- main conv loop ----
    mpsum = ctx.enter_context(tc.tile_pool(name="mpsum", bufs=7, space="PSUM"))
    spsum = ctx.enter_context(tc.tile_pool(name="spsum", bufs=1, space="PSUM"))
    opool = ctx.enter_context(tc.tile_pool(name="obuf", bufs=3))
    upool = ctx.enter_context(tc.tile_pool(name="ubuf", bufs=3))

    # one PSUM bank shared by the three tiny demod tensors:
    # sP [128,1] | sProw [1,C] at col 4 | demB [128,C] at col 132
    dt_ = spsum.tile([128, 132 + C], f32)

    # Process the short tail block right after the KSWITCH warmup blocks so
    # its eviction+DMA latency hides mid-kernel instead of tailing the run.
    i0s = list(range(0, H, R_BLK))
    i0_order = i0s[:KSWITCH] + [i0s[-1]] + i0s[KSWITCH:-1]
    for bi, i0 in enumerate(i0_order):
        R = min(R_BLK, H - i0)
        bt, lo = _band_for(i0, R)
        # the final block uses the 16-matmul path: its eviction is one
        # engine-level deep, trimming the end-of-kernel latency tail that
        # the 3-op hybrid assembly chain would add
        hybrid = bi >= KSWITCH and R == R_BLK and bi != len(i0_order) - 1
        ob = opool.tile([128, R, 2, 2 * W], bf16, tag="ob", name=f"ob_{i0}")
        obv = ob.rearrange("p r d (j two) -> p r d two j", two=2)

        if not hybrid:
            ph = []
            for p in range(4):
                di, dj = p >> 1, p & 1
                pt = mpsum.tile([128, R * W], f32, tag="ph", name=f"ph{p}_{i0}")
                for q in range(4):
                    a, b = q >> 1, q & 1
                    r0 = i0 + (a + di - 1) - lo      # tile row of first x row
                    co = b + dj - 1
                    rhs = bt[:, r0 : r0 + R, co + 1 : co + 1 + W]
                    nc.tensor.matmul(
                        pt[:], _conv_weight(di, dj, a, b), rhs,
                        start=(q == 0), stop=(q == 3),
                    )
                ph.append(pt)
            if bi == 0:
                # demod partition sums, queued behind block 0's matmuls:
                # sP[o,1] for the eviction scale, sProw[1,o] for the fold.
                sP = dt_[:, 0:1]
                nc.tensor.matmul(sP, s2[:], onesS[:], start=True, stop=True)
                sProw = dt_[0:1, 4 : 4 + C]
                nc.tensor.matmul(sProw, onesS[:], s2[:], start=True, stop=True)
                t1 = const.tile([128, 1], f32)
                nc.vector.tensor_scalar_add(t1[:], sP, EPS)
                t2 = const.tile([128, 1], f32)
                nc.scalar.sqrt(t2[:], t1[:])
                nc.vector.reciprocal(demP[:], t2[:])
                r1 = const.tile([1, C], f32)
                nc.vector.tensor_scalar_add(r1[:], sProw, EPS)
                r2 = const.tile([1, C], f32)
                nc.scalar.sqrt(r2[:], r1[:])
                demRow = const.tile([1, C], f32)
                nc.vector.reciprocal(demRow[:], r2[:])
            if bi == 2:
                # demB[p, o] = dem[o] on every partition (outer product with
                # a ones row); by now demRow is ready so the PE doesn't stall.
                demB = dt_[:, 132 : 132 + C]
                nc.tensor.matmul(demB, onesRow[:], demRow[:], start=True, stop=True)
            # interleave phases into output rows; scale by demod, add bias
            for p in range(4):
                di, dj = p >> 1, p & 1
                dst = obv[:, :, di, dj, :]
                srcv = ph[p].rearrange("p (r j) -> p r j", r=R)
                if dj == 0:
                    nc.vector.tensor_scalar(
                        dst, srcv, demP[:, 0:1], dmb[:, 1:2],
                        op0=ALU.mult, op1=ALU.add,
                    )
                else:
                    nc.scalar.activation(
                        dst, srcv, AF.Identity, bias=dmb[:, 1:2], scale=demP[:, 0:1]
                    )
            if bi == 2:
                # fold demod into the weights for the hybrid blocks
                demB = dt_[:, 132 : 132 + C]
                demBt = demB.unsqueeze(1).broadcast_to([128, 9, C])
                nc.vector.tensor_tensor(wm3F[:], wm3[:], demBt, op=ALU.mult)
                nc.vector.tensor_add(R01F[:], wm3F[:, 3:6, :], wm3F[:, 6:9, :])
                nc.vector.tensor_add(R10F[:], wm3F[:, 0:3, :], wm3F[:, 3:6, :])
                for a in range(2):
                    S = rowsrcF[(1, a)]
                    nc.vector.tensor_add(
                        comb4F[:, 2 * a, :], S[:, 1, :], S[:, 2, :]
                    )
                    nc.vector.tensor_add(
                        comb4F[:, 2 * a + 1, :], S[:, 0, :], S[:, 1, :]
                    )
        else:
            # ---- hybrid 14-matmul block ----
            # di=0: three column-tap tiles t[kj], 2 row-shift matmuls each
            tts = []
            for kj in range(3):
                tt = mpsum.tile([128, R, WP], f32, tag="ph", name=f"tt{kj}_{i0}")
                for a in range(2):
                    wgt = wm3F[:, kj, :] if a == 0 else R01F[:, kj, :]
                    r0 = i0 + a - 1 - lo
                    nc.tensor.matmul(
                        tt[:], wgt, bt[:, r0 : r0 + R, :],
                        start=(a == 0), stop=(a == 1),
                    )
                tts.append(tt)
            # di=1: direct 4-tap phases
            pts = []
            for dj in range(2):
                pt = mpsum.tile([128, R * W], f32, tag="ph", name=f"q{dj}_{i0}")
                for q in range(4):
                    a, b = q >> 1, q & 1
                    r0 = i0 + a - lo
                    co = b + dj - 1
                    rhs = bt[:, r0 : r0 + R, co + 1 : co + 1 + W]
                    nc.tensor.matmul(
                        pt[:], _conv_weightF(dj, a, b), rhs,
                        start=(q == 0), stop=(q == 3),
                    )
                pts.append(pt)
            # di=0 assembly:
            #   y[2i, 2j+d] = t0[j+d] + t1[j+1] + t2[j+1+d] + bias
            # Each op reads at most one PSUM operand (DVE single-PSUM-port):
            #   ACT: sA[j]     = t1[j+1] + bias
            #   DVE: u[j,d]    = t2[j+1+d] + sA[j]     (0-stride pair bcast)
            #   DVE: ob0[j,d]  = t0[j+d] + u[j,d]
            sA = upool.tile([128, R, W], bf16, tag="sA", name=f"sA_{i0}")
            nc.scalar.activation(
                sA[:], tts[1][:, :, 1 : 1 + W], AF.Identity, bias=dmb[:, 1:2]
            )
            u = upool.tile([128, R, 2 * W], bf16, tag="u", name=f"u_{i0}")
            uv = u.rearrange("p r (j d) -> p r j d", d=2)
            sA4 = sA[:].unsqueeze(-1).broadcast_to([128, R, W, 2])
            nc.vector.tensor_tensor(uv, _pair_view(tts[2], 1), sA4, op=ALU.add)
            ob0v = ob[:, :, 0, :].rearrange("p r (j d) -> p r j d", d=2)
            nc.vector.tensor_tensor(ob0v, _pair_view(tts[0], 0), uv, op=ALU.add)
            # di=1 evictions on ACT, bias only (demod already in weights)
            for dj in range(2):
                dst = obv[:, :, 1, dj, :]
                srcv = pts[dj].rearrange("p (r j) -> p r j", r=R)
                nc.scalar.activation(dst, srcv, AF.Identity, bias=dmb[:, 1:2])

        nc.sync.dma_start(y[:, 2 * i0 : 2 * i0 + 2 * R, :], ob[:])


def _build():
    nc = bacc.Bacc(
        "TRN2",
        target_bir_lowering=False,
        debug=False,
        enable_asserts=False,
        num_devices=NCORES,
    )
    x = nc.dram_tensor("x", [C, H + 2, W + 2], bf16, kind="ExternalInput").ap()
    dmbias = nc.dram_tensor("dmbias", [2, C], f32, kind="ExternalInput").ap()
    wbT = nc.dram_tensor("WbT", [C, 9 * C], bf16, kind="ExternalInput").ap()
    luT = nc.dram_tensor("lora_upT", [RANK, C], bf16, kind="ExternalInput").ap()
    ldT = nc.dram_tensor("lora_downT", [RANK, 9 * C], bf16, kind="ExternalInput").ap()
    ident2 = nc.dram_tensor("ident2", [2, 2], f32, kind="ExternalInput").ap()
    y = nc.dram_tensor("y", [C, 2 * H, 2 * W], bf16, kind="ExternalOutput").ap()

    with tile.TileContext(nc) as tc:
        with ExitStack() as ctx:
            _conv_kernel(ctx, tc, y, x, dmbias, wbT, luT, ldT, ident2)
    nc.compile()
    return nc


_CACHE = {}


def _get_nc():
    if "nc" not in _CACHE:
        _CACHE["nc"] = _build()
    return _CACHE["nc"]


def _make_in_maps(x, de_mod, Wb, lora_up, lora_down, bias):
    bf = ml_dtypes.bfloat16
    x = np.asarray(x, dtype=np.float32).astype(bf)
    # zero-pad the spatial borders on the host: the band DMAs then deliver
    # halo rows/columns directly (layout-only prep)
    xp = np.zeros((B, C, H + 2, W + 2), dtype=bf)
    xp[:, :, 1 : H + 1, 1 : W + 1] = x
    de_mod = np.asarray(de_mod, dtype=np.float32)
    Wb = np.asarray(Wb, dtype=np.float32)
    lora_up = np.asarray(lora_up, dtype=np.float32)
    lora_down = np.asarray(lora_down, dtype=np.float32)
    # layout-only host prep: [O,I,3,3] -> [i, (t o)], [R,C,3,3] -> [r, (t i)]
    wbT = np.ascontiguousarray(Wb.transpose(1, 2, 3, 0).reshape(C, 9 * C)).astype(bf)
    luT = np.ascontiguousarray(lora_up.T).astype(bf)
    ldT = np.ascontiguousarray(
        lora_down.transpose(0, 2, 3, 1).reshape(RANK, 9 * C)
    ).astype(bf)
    bias = np.asarray(bias, dtype=np.float32).reshape(C)
    id2 = np.eye(2, dtype=np.float32)
    in_maps = []
    for b in range(NCORES):
        in_maps.append(
            {
                "x": np.ascontiguousarray(xp[b]),
                "dmbias": np.ascontiguousarray(np.stack([de_mod[b], bias])),
                "WbT": wbT,
                "lora_upT": luT,
                "lora_downT": ldT,
                "ident2": id2,
            }
        )
    return in_maps


def run(inputs, trace=False, trace_kwargs=None):
    nc = _get_nc()
    in_maps = _make_in_maps(**inputs)
    res = run_bass_kernel_spmd(
        nc,
        in_maps,
        core_ids=list(range(NCORES)),
        trace=trace,
        **(trace_kwargs or {}),
    )
    y = np.stack(
        [res.results[b]["y"].astype(np.float32) for b in range(NCORES)], axis=0
    )
    return y, res


def kernel(**inputs):
    y, _ = run(inputs)
    return y


# revision 29
# speedup vs baseline: 1.0042x; 1.0042x over previous
"""Trainium2 Bass kernel for nn_NeuronS3DiffUpsample2D.

Reference computation (per sample b):
    up   = nearest-2x-upsample(x[b])                       # [C, 320, 320]
    w    = Wb + 0.25 * einsum('or,rikl->oikl', lora_up, lora_down)
    w_b  = w * de_mod[b, None, :, None, None]              # modulate input chans
    dem  = rsqrt(sum_{i,k,l} w_b^2 + eps)                  # per output chan
    y[b] = conv2d(up, w_b * dem, SAME) + bias

Key algebraic transform: a 3x3 SAME conv on a 2x nearest-upsampled image
decomposes into 4 output phases (di, dj in {0,1}), each a 2x2 conv on the
ORIGINAL 160x160 input:
    y[2i+di, 2j+dj] = sum_{a,b in {0,1}} K[di,dj,a,b] @ x[i+a+di-1, j+b+dj-1]
where the 16 [O, I] matrices K are sums of 1/2/4 of the 9 taps of w.
This is 4/9 of the naive FLOPs and never materializes the upsampled image.

On top of that, the steady-state loop uses a 14-matmul block: the two di=0
phases share three column-tap tiles t[kj][i,j] = sum_a w[S(0,a),kj] (x) at
row shifts (6 matmuls, PE) and are assembled by the DVE as
    y[2i, 2j+dj] = t0[j+dj] + t1[j+1] + t2[j+1+dj] + bias
(shifted overlapping-pair access patterns; t1 uses a 0-stride broadcast
pair).  The di=1 phases stay as direct 4-tap accumulations (8 matmuls)
evicted by the ACT engine.  This trades 2 of 16 PE matmuls per block for
DVE/ACT work that fits in their idle capacity: PE is the only saturated
engine (~90% busy at 1 col/cycle).

For the assembly to be a pure add, the demod scale is folded INTO the
weights.  demod depends on the weights themselves, so the first KSWITCH
blocks run the original 16-matmul path (demod applied per-partition at PSUM
eviction) while the fold chain (row-form demod via a [1,C] PE reduction, a
broadcast outer product, and a DVE multiply) completes in the background.

Sharding: data-parallel over batch B=8 across 8 NeuronCores; each core builds
its own per-sample weights locally.  Host-side work is layout only (slicing,
transposition, fp32->bf16 rounding).  All arithmetic is on device.

Everything runs in bf16 (PE streams bf16 at 1 cycle/row like f32r, but
LDWEIGHTS fully hides under the previous 480-col stream, and input/output
DMA halves; measured rel err 4e-3 vs the 2e-2 budget).  Output goes to DRAM
as bf16 and is widened to f32 on the host.

The input arrives host-padded ([C, 162, 162] with zero borders) so a single
contiguous DMA per band delivers data and halos together — on-device border
zero-writes race with the unaligned bf16 DMA's write granules (observed as
nondeterministic right-edge outputs).  The first band is short so the first
conv matmul isn't gated behind a 1.2MB transfer, and the tail block is
processed early so its eviction+DMA latency hides mid-kernel.  Weight DMAs
are split across the sync and ACT queues; bands ride the gpsimd SWDGE queue.

Measured on 8 axon-tunneled TRN2 cores: 186.0us (vs 225.6us baseline) at
the PE stream roofline (204.5ns per 480-col bf16 matmul, Tensor ~90% busy);
run-to-run device clock variance is ~±10%.
"""

import sys
import numpy as np
import ml_dtypes
from contextlib import ExitStack

try:
    import concourse.bass as bass
except ImportError:  # grading env without the axon PYTHONPATH
    sys.path.insert(0, "/opt/trn_rl_repo")
    import concourse.bass as bass
import concourse.tile as tile
from concourse import bacc, mybir
from concourse.bass_utils import run_bass_kernel_spmd

B, C, H, W = 8, 128, 160, 160
RANK = 32
SCALING = 0.25
EPS = 1e-8
WP = W + 2          # padded row length (zero col on each side)
R_BLK = 3           # x-rows per matmul block -> N = 3*160 = 480 <= 512
# band cut points: short first band so block 0 starts early
CUTS = [0, 15, 42, 69, 96, 123, 150, 160]
KSWITCH = 5         # blocks on the 16-matmul path while demod-fold completes
NCORES = 8

f32 = mybir.dt.float32
bf16 = mybir.dt.bfloat16


def _conv_kernel(ctx, tc, y, x, dmbias, wbT, luT, ldT, ident2):
    nc = tc.nc
    AF = mybir.ActivationFunctionType
    ALU = mybir.AluOpType
    AX = mybir.AxisListType

    const = ctx.enter_context(tc.tile_pool(name="const", bufs=1))
    bands = ctx.enter_context(tc.tile_pool(name="bands", bufs=7))

    # dmbias/ident2 (tiny, gate the de_mod transpose) on sync; weight
    # tensors on the otherwise-idle ACT queue; bands on gpsimd SWDGE.
    dmbR = const.tile([2, C], f32)
    nc.sync.dma_start(dmbR[:], dmbias[:])
    id2 = const.tile([2, 2], f32)
    nc.sync.dma_start(id2[:], ident2[:])
    # WbT split across the sync and ACT queues so the two halves transfer
    # in parallel (a single 295KB DMA gated the whole weight chain)
    WbTS = const.tile([128, 9, C], bf16)         # Wb^T: [i, t, o]
    nc.sync.dma_start(WbTS[:, 0:5, :], wbT[:, 0 : 5 * C])
    LD9 = const.tile([RANK, 9, C], bf16)         # lora_down^T: [r, t, i]
    nc.scalar.dma_start(LD9[:], ldT[:])
    LUTn = const.tile([RANK, C], bf16)           # lora_up^T: [r, o]
    nc.scalar.dma_start(LUTn[:], luT[:])
    nc.scalar.dma_start(WbTS[:, 5:9, :], wbT[:, 5 * C : 9 * C])

    # weight tensors the conv loop reads as stationary operands
    wm3 = const.tile([128, 9, C], bf16)          # modulated w^T: [i, t, o]
    R01 = const.tile([128, 3, C], bf16)          # rows ki1+ki2
    R10 = const.tile([128, 3, C], bf16)          # rows ki0+ki1
    comb8 = const.tile([128, 8, C], bf16)        # two-column tap sums (unfolded)
    wm3F = const.tile([128, 9, C], bf16)         # demod-folded variants
    R01F = const.tile([128, 3, C], bf16)
    R10F = const.tile([128, 3, C], bf16)
    comb4F = const.tile([128, 4, C], bf16)       # folded di=1 two-column sums
    demP = const.tile([128, 1], f32)             # rsqrt demod, per output chan
    dmb = const.tile([128, 3], f32)              # de_mod[i], bias[o], 0.25*de_mod
    s2 = const.tile([128, C], f32)               # per-(i,o) tap-summed squares
    onesS = const.tile([128, 1], f32)
    onesRow = const.tile([1, C], f32)

    nc.vector.memset(onesS[:], 1.0)
    nc.vector.memset(onesRow[:], 1.0)

    # ---- input bands: (lo, hi) are halo-inclusive x-row bounds.  x arrives
    # pre-padded on the host ([C, H+2, W+2] with zero borders), so a single
    # whole-tile contiguous DMA delivers data AND halos: no on-device border
    # writes (a disjoint border zero-write races with the DMA's write
    # granules on HW), and per-partition descriptors coalesce.
    segs = [(CUTS[i] - 1, min(CUTS[i + 1], H)) for i in range(len(CUTS) - 1)]
    # DMA issue order: bands 0 and 1 first (consumed earliest), then the
    # LAST band (the tail block is processed early, see below), then the
    # rest in order.
    dma_order = [0, 1, len(segs) - 1] + list(range(2, len(segs) - 1))
    band_tiles = [None] * len(segs)
    for si in dma_order:
        lo, hi = segs[si]
        nrows = hi - lo + 1
        bt = bands.tile([128, nrows, WP], bf16, tag="band", name=f"band{lo}")
        nc.gpsimd.dma_start(bt[:], x[:, lo + 1 : hi + 2, :])
        band_tiles[si] = (bt, lo, hi)

    def _band_for(i0, R):
        for bt, lo, hi in band_tiles:
            if lo <= i0 - 1 and i0 + R <= hi:
                return bt, lo
        raise AssertionError(f"no band for block {i0}")

    # ---- weight stage ----
    with tc.tile_pool(name="wtmp", bufs=1) as wtmp, tc.tile_pool(
        name="wpsum", bufs=1, space="PSUM"
    ) as wpsum:
        dmbP = wpsum.tile([128, 2], f32)
        nc.tensor.transpose(dmbP[:], dmbR[:], id2[:])
        nc.vector.tensor_copy(dmb[:, 0:2], dmbP[:])
        nc.vector.tensor_scalar_mul(dmb[:, 2:3], dmb[:, 0:1], SCALING)

        # deltaT_unscaled[i, t, o] = sum_r down[r,i,t] * up[o,r]; the 0.25
        # lora scale rides in via the fused modulation below
        deltaP = wpsum.tile([128, 9, C], f32)
        for t in range(9):
            nc.tensor.matmul(
                deltaP[:, t, :], LD9[:, t, :], LUTn[:], start=True, stop=True
            )

        # wm3 = Wb^T*dm + deltaT*(0.25*dm)
        WbTm = wtmp.tile([128, 9, C], bf16)
        nc.vector.tensor_scalar_mul(WbTm[:], WbTS[:], dmb[:, 0:1])
        nc.vector.scalar_tensor_tensor(
            wm3[:], deltaP[:], dmb[:, 2:3], WbTm[:],
            op0=ALU.mult, op1=ALU.add,
        )

        # Row combos over ki (t = 3*ki + kj):
        #   (di=0, a=0): ki0        (di=0, a=1): ki1+ki2
        #   (di=1, a=0): ki0+ki1    (di=1, a=1): ki2
        # Single-column taps are read directly out of wm3/R01/R10; only the
        # two-column sums are materialized, phase-0 slots first.
        nc.vector.tensor_add(R01[:], wm3[:, 3:6, :], wm3[:, 6:9, :])
        nc.vector.tensor_add(R10[:], wm3[:, 0:3, :], wm3[:, 3:6, :])
        rowsrc = {
            (0, 0): wm3[:, 0:3, :],
            (0, 1): R01[:],
            (1, 0): R10[:],
            (1, 1): wm3[:, 6:9, :],
        }
        for p in range(4):
            di, dj = p >> 1, p & 1
            for a in range(2):
                S = rowsrc[(di, a)]
                dst = comb8[:, 4 * di + 2 * a + dj, :]
                if dj == 0:      # (dj=0, b=1): kj1 + kj2
                    nc.vector.tensor_add(dst, S[:, 1, :], S[:, 2, :])
                else:            # (dj=1, b=0): kj0 + kj1
                    nc.vector.tensor_add(dst, S[:, 0, :], S[:, 1, :])

        # demod source: sq3 = wm3^2 (ACT), tap-sum on DVE.  The partition
        # sums (PE matmuls) are emitted inside the conv loop so the in-order
        # tensor queue doesn't stall the conv behind this reduce.
        sq3 = wtmp.tile([128, 9, C], f32)
        nc.scalar.square(sq3[:], wm3[:])
        nc.vector.tensor_reduce(
            s2[:], sq3.rearrange("p t o -> p o t"), axis=AX.X, op=ALU.add
        )

    def _conv_weight(di, dj, a, b):
        if dj == 0 and b == 0:
            return rowsrc[(di, a)][:, 0, :]
        if dj == 1 and b == 1:
            return rowsrc[(di, a)][:, 2, :]
        return comb8[:, 4 * di + 2 * a + dj, :]

    rowsrcF = {(1, 0): R10F[:], (1, 1): wm3F[:, 6:9, :]}

    def _conv_weightF(dj, a, b):  # di=1 only
        if dj == 0 and b == 0:
            return rowsrcF[(1, a)][:, 0, :]
        if dj == 1 and b == 1:
            return rowsrcF[(1, a)][:, 2, :]
        return comb4F[:, 2 * a + dj, :]

    def _pair_view(tt, col0):
        """[128, R, W, 2] view of a [128, R, WP] tile: (j, d) -> col j+d+col0."""
        ap = tt[:].copy()
        ap.ap = ap.ap[:-1] + [[1, W], [1, 2]]
        ap.offset = ap.offset + col0
        return ap

    # ---- main conv loop ----
    mpsum = ctx.enter_context(tc.tile_pool(name="mpsum", bufs=7, space="PSUM"))
    spsum = ctx.enter_context(tc.tile_pool(name="spsum", bufs=1, space="PSUM"))
    opool = ctx.enter_context(tc.tile_pool(name="obuf", bufs=3))
    upool = ctx.enter_context(tc.tile_pool(name="ubuf", bufs=3))

    # one PSUM bank shared by the three tiny demod tensors:
    # sP [128,1] | sProw [1,C] at col 4 | demB [128,C] at col 132
    dt_ = spsum.tile([128, 132 + C], f32)

    # Process the short tail block right after the KSWITCH warmup blocks so
    # its eviction+DMA latency hides mid-kernel instead of tailing the run.
    i0s = list(range(0, H, R_BLK))
    i0_order = i0s[:KSWITCH] + [i0s[-1]] + i0s[KSWITCH:-1]
    for bi, i0 in enumerate(i0_order):
        R = min(R_BLK, H - i0)
        bt, lo = _band_for(i0, R)
        # the final block uses the 16-matmul path: its eviction is one
        # engine-level deep, trimming the end-of-kernel latency tail that
        # the 3-op hybrid assembly chain would add
        hybrid = bi >= KSWITCH and R == R_BLK and bi != len(i0_order) - 1
        ob = opool.tile([128, R, 2, 2 * W], bf16, tag="ob", name=f"ob_{i0}")
        obv = ob.rearrange("p r d (j two) -> p r d two j", two=2)

        if not hybrid:
            ph = []
            for p in range(4):
                di, dj = p >> 1, p & 1
                pt = mpsum.tile([128, R * W], f32, tag="ph", name=f"ph{p}_{i0}")
                for q in range(4):
                    a, b = q >> 1, q & 1
                    r0 = i0 + (a + di - 1) - lo      # tile row of first x row
                    co = b + dj - 1
                    rhs = bt[:, r0 : r0 + R, co + 1 : co + 1 + W]
                    nc.tensor.matmul(
                        pt[:], _conv_weight(di, dj, a, b), rhs,
                        start=(q == 0), stop=(q == 3),
                    )
                ph.append(pt)
            if bi == 0:
                # demod partition sums, queued behind block 0's matmuls:
                # sP[o,1] for the eviction scale, sProw[1,o] for the fold.
                sP = dt_[:, 0:1]
                nc.tensor.matmul(sP, s2[:], onesS[:], start=True, stop=True)
                sProw = dt_[0:1, 4 : 4 + C]
                nc.tensor.matmul(sProw, onesS[:], s2[:], start=True, stop=True)
                t1 = const.tile([128, 1], f32)
                nc.vector.tensor_scalar_add(t1[:], sP, EPS)
                t2 = const.tile([128, 1], f32)
                nc.scalar.sqrt(t2[:], t1[:])
                nc.vector.reciprocal(demP[:], t2[:])
                r1 = const.tile([1, C], f32)
                nc.vector.tensor_scalar_add(r1[:], sProw, EPS)
                r2 = const.tile([1, C], f32)
                nc.scalar.sqrt(r2[:], r1[:])
                demRow = const.tile([1, C], f32)
                nc.vector.reciprocal(demRow[:], r2[:])
            if bi == 2:
                # demB[p, o] = dem[o] on every partition (outer product with
                # a ones row); by now demRow is ready so the PE doesn't stall.
                demB = dt_[:, 132 : 132 + C]
                nc.tensor.matmul(demB, onesRow[:], demRow[:], start=True, stop=True)
            # interleave phases into output rows; scale by demod, add bias
            for p in range(4):
                di, dj = p >> 1, p & 1
                dst = obv[:, :, di, dj, :]
                srcv = ph[p].rearrange("p (r j) -> p r j", r=R)
                if dj == 0:
                    nc.vector.tensor_scalar(
                        dst, srcv, demP[:, 0:1], dmb[:, 1:2],
                        op0=ALU.mult, op1=ALU.add,
                    )
                else:
                    nc.scalar.activation(
                        dst, srcv, AF.Identity, bias=dmb[:, 1:2], scale=demP[:, 0:1]
                    )
            if bi == 2:
                # fold demod into the weights for the hybrid blocks
                demB = dt_[:, 132 : 132 + C]
                demBt = demB.unsqueeze(1).broadcast_to([128, 9, C])
                nc.vector.tensor_tensor(wm3F[:], wm3[:], demBt, op=ALU.mult)
                nc.vector.tensor_add(R01F[:], wm3F[:, 3:6, :], wm3F[:, 6:9, :])
                nc.vector.tensor_add(R10F[:], wm3F[:, 0:3, :], wm3F[:, 3:6, :])
                for a in range(2):
                    S = rowsrcF[(1, a)]
                    nc.vector.tensor_add(
                        comb4F[:, 2 * a, :], S[:, 1, :], S[:, 2, :]
                    )
                    nc.vector.tensor_add(
                        comb4F[:, 2 * a + 1, :], S[:, 0, :], S[:, 1, :]
                    )
        else:
            # ---- hybrid 14-matmul block ----
            # di=0: three column-tap tiles t[kj], 2 row-shift matmuls each
            tts = []
            for kj in range(3):
                tt = mpsum.tile([128, R, WP], f32, tag="ph", name=f"tt{kj}_{i0}")
                for a in range(2):
                    wgt = wm3F[:, kj, :] if a == 0 else R01F[:, kj, :]
                    r0 = i0 + a - 1 - lo
                    nc.tensor.matmul(
                        tt[:], wgt, bt[:, r0 : r0 + R, :],
                        start=(a == 0), stop=(a == 1),
                    )
                tts.append(tt)
            # di=1: direct 4-tap phases
            pts = []
            for dj in range(2):
                pt = mpsum.tile([128, R * W], f32, tag="ph", name=f"q{dj}_{i0}")
                for q in range(4):
                    a, b = q >> 1, q & 1
                    r0 = i0 + a - lo
                    co = b + dj - 1
                    rhs = bt[:, r0 : r0 + R, co + 1 : co + 1 + W]
                    nc.tensor.matmul(
                        pt[:], _conv_weightF(dj, a, b), rhs,
                        start=(q == 0), stop=(q == 3),
                    )
                pts.append(pt)
            # di=0 assembly:
            #   y[2i, 2j+d] = t0[j+d] + t1[j+1] + t2[j+1+d] + bias
            # Each op reads at most one PSUM operand (DVE single-PSUM-port):
            #   ACT: sA[j]     = t1[j+1] + bias
            #   DVE: u[j,d]    = t2[j+1+d] + sA[j]     (0-stride pair bcast)
            #   DVE: ob0[j,d]  = t0[j+d] + u[j,d]
            sA = upool.tile([128, R, W], bf16, tag="sA", name=f"sA_{i0}")
            nc.scalar.activation(
                sA[:], tts[1][:, :, 1 : 1 + W], AF.Identity, bias=dmb[:, 1:2]
            )
            u = upool.tile([128, R, 2 * W], bf16, tag="u", name=f"u_{i0}")
            uv = u.rearrange("p r (j d) -> p r j d", d=2)
            sA4 = sA[:].unsqueeze(-1).broadcast_to([128, R, W, 2])
            nc.vector.tensor_tensor(uv, _pair_view(tts[2], 1), sA4, op=ALU.add)
            ob0v = ob[:, :, 0, :].rearrange("p r (j d) -> p r j d", d=2)
            nc.vector.tensor_tensor(ob0v, _pair_view(tts[0], 0), uv, op=ALU.add)
            # di=1 evictions on ACT, bias only (demod already in weights)
            for dj in range(2):
                dst = obv[:, :, 1, dj, :]
                srcv = pts[dj].rearrange("p (r j) -> p r j", r=R)
                nc.scalar.activation(dst, srcv, AF.Identity, bias=dmb[:, 1:2])

        nc.sync.dma_start(y[:, 2 * i0 : 2 * i0 + 2 * R, :], ob[:])


def _build():
    nc = bacc.Bacc(
        "TRN2",
        target_bir_lowering=False,
        debug=False,
        enable_asserts=False,
        num_devices=NCORES,
    )
    x = nc.dram_tensor("x", [C, H + 2, W + 2], bf16, kind="ExternalInput").ap()
    dmbias = nc.dram_tensor("dmbias", [2, C], f32, kind="ExternalInput").ap()
    wbT = nc.dram_tensor("WbT", [C, 9 * C], bf16, kind="ExternalInput").ap()
    luT = nc.dram_tensor("lora_upT", [RANK, C], bf16, kind="ExternalInput").ap()
    ldT = nc.dram_tensor("lora_downT", [RANK, 9 * C], bf16, kind="ExternalInput").ap()
    ident2 = nc.dram_tensor("ident2", [2, 2], f32, kind="ExternalInput").ap()
    y = nc.dram_tensor("y", [C, 2 * H, 2 * W], bf16, kind="ExternalOutput").ap()

    with tile.TileContext(nc) as tc:
        with ExitStack() as ctx:
            _conv_kernel(ctx, tc, y, x, dmbias, wbT, luT, ldT, ident2)
    nc.compile()
    return nc


_CACHE = {}


def _get_nc():
    if "nc" not in _CACHE:
        _CACHE["nc"] = _build()
    return _CACHE["nc"]


def _make_in_maps(x, de_mod, Wb, lora_up, lora_down, bias):
    bf = ml_dtypes.bfloat16
    x = np.asarray(x, dtype=np.float32).astype(bf)
    # zero-pad the spatial borders on the host: the band DMAs then deliver
    # halo rows/columns directly (layout-only prep)
    xp = np.zeros((B, C, H + 2, W + 2), dtype=bf)
    xp[:, :, 1 : H + 1, 1 : W + 1] = x
    de_mod = np.asarray(de_mod, dtype=np.float32)
    Wb = np.asarray(Wb, dtype=np.float32)
    lora_up = np.asarray(lora_up, dtype=np.float32)
    lora_down = np.asarray(lora_down, dtype=np.float32)
    # layout-only host prep: [O,I,3,3] -> [i, (t o)], [R,C,3,3] -> [r, (t i)]
    wbT = np.ascontiguousarray(Wb.transpose(1, 2, 3, 0).reshape(C, 9 * C)).astype(bf)
    luT = np.ascontiguousarray(lora_up.T).astype(bf)
    ldT = np.ascontiguousarray(
        lora_down.transpose(0, 2, 3, 1).reshape(RANK, 9 * C)
    ).astype(bf)
    bias = np.asarray(bias, dtype=np.float32).reshape(C)
    id2 = np.eye(2, dtype=np.float32)
    in_maps = []
    for b in range(NCORES):
        in_maps.append(
            {
                "x": np.ascontiguousarray(xp[b]),
                "dmbias": np.ascontiguousarray(np.stack([de_mod[b], bias])),
                "WbT": wbT,
                "lora_upT": luT,
                "lora_downT": ldT,
                "ident2": id2,
            }
        )
    return in_maps


def run(inputs, trace=False, trace_kwargs=None):
    nc = _get_nc()
    in_maps = _make_in_maps(**inputs)
    res = run_bass_kernel_spmd(
        nc,
        in_maps,
        core_ids=list(range(NCORES)),
        trace=trace,
        **(trace_kwargs or {}),
    )
    y = np.stack(
        [res.results[b]["y"].astype(np.float32) for b in range(NCORES)], axis=0
    )
    return y, res


def kernel(**inputs):
    y, _ = run(inputs)
    return y


# revision 32
# speedup vs baseline: 1.1650x; 1.1602x over previous
"""Trainium2 Bass kernel for nn_NeuronS3DiffUpsample2D.

Reference computation (per sample b):
    up   = nearest-2x-upsample(x[b])                       # [C, 320, 320]
    w    = Wb + 0.25 * einsum('or,rikl->oikl', lora_up, lora_down)
    w_b  = w * de_mod[b, None, :, None, None]              # modulate input chans
    dem  = rsqrt(sum_{i,k,l} w_b^2 + eps)                  # per output chan
    y[b] = conv2d(up, w_b * dem, SAME) + bias

Key algebraic transform: a 3x3 SAME conv on a 2x nearest-upsampled image
decomposes into 4 output phases (di, dj in {0,1}), each a 2x2 conv on the
ORIGINAL 160x160 input:
    y[2i+di, 2j+dj] = sum_{a,b in {0,1}} K[di,dj,a,b] @ x[i+a+di-1, j+b+dj-1]
where the 16 [O, I] matrices K are sums of 1/2/4 of the 9 taps of w.
This is 4/9 of the naive FLOPs and never materializes the upsampled image.

On top of that, the steady-state loop uses a 14-matmul block: the two di=0
phases share three column-tap tiles t[kj][i,j] = sum_a w[S(0,a),kj] (x) at
row shifts (6 matmuls, PE) and are assembled by the DVE as
    y[2i, 2j+dj] = t0[j+dj] + t1[j+1] + t2[j+1+dj] + bias
(shifted overlapping-pair access patterns; t1 uses a 0-stride broadcast
pair).  The di=1 phases stay as direct 4-tap accumulations (8 matmuls)
evicted by the ACT engine.  This trades 2 of 16 PE matmuls per block for
DVE/ACT work that fits in their idle capacity: PE is the only saturated
engine (~90% busy at 1 col/cycle).

For the assembly to be a pure add, the demod scale is folded INTO the
weights.  demod depends on the weights themselves, so the first KSWITCH
blocks run the original 16-matmul path (demod applied per-partition at PSUM
eviction) while the fold chain (row-form demod via a [1,C] PE reduction, a
broadcast outer product, and a DVE multiply) completes in the background.

Sharding: data-parallel over batch B=8 across 8 NeuronCores; each core builds
its own per-sample weights locally.  Host-side work is layout only (slicing,
transposition, fp32->bf16 rounding).  All arithmetic is on device.

Everything runs in bf16 (PE streams bf16 at 1 cycle/row like f32r, but
LDWEIGHTS fully hides under the previous 480-col stream, and input/output
DMA halves; measured rel err 4e-3 vs the 2e-2 budget).  Output goes to DRAM
as bf16 and is widened to f32 on the host.

The input arrives host-padded ([C, 162, 162] with zero borders) so a single
contiguous DMA per band delivers data and halos together — on-device border
zero-writes race with the unaligned bf16 DMA's write granules (observed as
nondeterministic right-edge outputs).  The first band is short so the first
conv matmul isn't gated behind a 1.2MB transfer, and the tail block is
processed early so its eviction+DMA latency hides mid-kernel.  Weight DMAs
are split across the sync and ACT queues; bands ride the gpsimd SWDGE queue.

Measured on 8 axon-tunneled TRN2 cores: 186.0us (vs 225.6us baseline) at
the PE stream roofline (204.5ns per 480-col bf16 matmul, Tensor ~90% busy);
run-to-run device clock variance is ~±10%.
"""

import sys
import numpy as np
import ml_dtypes
from contextlib import ExitStack

try:
    import concourse.bass as bass
except ImportError:  # grading env without the axon PYTHONPATH
    sys.path.insert(0, "/opt/trn_rl_repo")
    import concourse.bass as bass
import concourse.tile as tile
from concourse import bacc, mybir
from concourse.bass_utils import run_bass_kernel_spmd

B, C, H, W = 8, 128, 160, 160
RANK = 32
SCALING = 0.25
EPS = 1e-8
WP = W + 2          # padded row length (zero col on each side)
R_BLK = 3           # x-rows per matmul block -> N = 3*160 = 480 <= 512
# band cut points: short first band so block 0 starts early
CUTS = [0, 15, 42, 69, 96, 123, 150, 160]
KSWITCH = 4         # blocks on the 16-matmul path while demod-fold completes
NCORES = 8

f32 = mybir.dt.float32
bf16 = mybir.dt.bfloat16


def _conv_kernel(ctx, tc, y, x, dmbias, wbT, luT, ldT, ident2):
    nc = tc.nc
    AF = mybir.ActivationFunctionType
    ALU = mybir.AluOpType
    AX = mybir.AxisListType

    const = ctx.enter_context(tc.tile_pool(name="const", bufs=1))
    bands = ctx.enter_context(tc.tile_pool(name="bands", bufs=7))

    # dmbias/ident2 (tiny, gate the de_mod transpose) on sync; weight
    # tensors on the otherwise-idle ACT queue; bands on gpsimd SWDGE.
    dmbR = const.tile([2, C], f32)
    nc.sync.dma_start(dmbR[:], dmbias[:])
    id2 = const.tile([2, 2], f32)
    nc.sync.dma_start(id2[:], ident2[:])
    # WbT split across the sync and ACT queues so the two halves transfer
    # in parallel (a single 295KB DMA gated the whole weight chain)
    # WbT split in three so the chunks transfer in parallel on the sync,
    # ACT and gpsimd queues (a single 295KB DMA gated the weight chain)
    WbTS = const.tile([128, 9, C], bf16)         # Wb^T: [i, t, o]
    nc.sync.dma_start(WbTS[:, 0:3, :], wbT[:, 0 : 3 * C])
    LD9 = const.tile([RANK, 9, C], bf16)         # lora_down^T: [r, t, i]
    nc.scalar.dma_start(LD9[:], ldT[:])
    LUTn = const.tile([RANK, C], bf16)           # lora_up^T: [r, o]
    nc.scalar.dma_start(LUTn[:], luT[:])
    nc.scalar.dma_start(WbTS[:, 3:6, :], wbT[:, 3 * C : 6 * C])
    nc.gpsimd.dma_start(WbTS[:, 6:9, :], wbT[:, 6 * C : 9 * C])

    # weight tensors the conv loop reads as stationary operands
    wm3 = const.tile([128, 9, C], bf16)          # modulated w^T: [i, t, o]
    R01 = const.tile([128, 3, C], bf16)          # rows ki1+ki2
    R10 = const.tile([128, 3, C], bf16)          # rows ki0+ki1
    comb8 = const.tile([128, 8, C], bf16)        # two-column tap sums (unfolded)
    wm3F = const.tile([128, 9, C], bf16)         # demod-folded variants
    R01F = const.tile([128, 3, C], bf16)
    R10F = const.tile([128, 3, C], bf16)
    comb4F = const.tile([128, 4, C], bf16)       # folded di=1 two-column sums
    demP = const.tile([128, 1], f32)             # rsqrt demod, per output chan
    dmb = const.tile([128, 3], f32)              # de_mod[i], bias[o], 0.25*de_mod
    s2 = const.tile([128, C], f32)               # per-(i,o) tap-summed squares
    onesS = const.tile([128, 1], f32)
    onesRow = const.tile([1, C], f32)

    nc.vector.memset(onesS[:], 1.0)
    nc.vector.memset(onesRow[:], 1.0)

    # ---- input bands: (lo, hi) are halo-inclusive x-row bounds.  x arrives
    # pre-padded on the host ([C, H+2, W+2] with zero borders), so a single
    # whole-tile contiguous DMA delivers data AND halos: no on-device border
    # writes (a disjoint border zero-write races with the DMA's write
    # granules on HW), and per-partition descriptors coalesce.
    segs = [(CUTS[i] - 1, min(CUTS[i + 1], H)) for i in range(len(CUTS) - 1)]
    # DMA issue order: bands 0 and 1 first (consumed earliest), then the
    # LAST band (the tail block is processed early, see below), then the
    # rest in order.
    dma_order = [0, 1, len(segs) - 1] + list(range(2, len(segs) - 1))
    band_tiles = [None] * len(segs)
    for si in dma_order:
        lo, hi = segs[si]
        nrows = hi - lo + 1
        bt = bands.tile([128, nrows, WP], bf16, tag="band", name=f"band{lo}")
        nc.gpsimd.dma_start(bt[:], x[:, lo + 1 : hi + 2, :])
        band_tiles[si] = (bt, lo, hi)

    def _band_for(i0, R):
        for bt, lo, hi in band_tiles:
            if lo <= i0 - 1 and i0 + R <= hi:
                return bt, lo
        raise AssertionError(f"no band for block {i0}")

    # ---- weight stage ----
    with tc.tile_pool(name="wtmp", bufs=1) as wtmp, tc.tile_pool(
        name="wpsum", bufs=1, space="PSUM"
    ) as wpsum:
        dmbP = wpsum.tile([128, 2], f32)
        nc.tensor.transpose(dmbP[:], dmbR[:], id2[:])
        nc.vector.tensor_copy(dmb[:, 0:2], dmbP[:])
        nc.vector.tensor_scalar_mul(dmb[:, 2:3], dmb[:, 0:1], SCALING)

        # deltaT_unscaled[i, t, o] = sum_r down[r,i,t] * up[o,r]; the 0.25
        # lora scale rides in via the fused modulation below
        deltaP = wpsum.tile([128, 9, C], f32)
        for t in range(9):
            nc.tensor.matmul(
                deltaP[:, t, :], LD9[:, t, :], LUTn[:], start=True, stop=True
            )

        # wm3 = Wb^T*dm + deltaT*(0.25*dm), in three tap-slices matching the
        # three WbT DMA chunks so the modulation chain starts on the first
        # bytes to land; the slices the first conv phase reads come first.
        # Row combos over ki (t = 3*ki + kj):
        #   (di=0, a=0): ki0        (di=0, a=1): ki1+ki2
        #   (di=1, a=0): ki0+ki1    (di=1, a=1): ki2
        # Single-column taps are read directly out of wm3/R01/R10; only the
        # two-column sums are materialized (slot = 4*di + 2*a + dj).
        WbTm = wtmp.tile([128, 9, C], bf16)

        def _mod_slice(sl):
            nc.vector.tensor_scalar_mul(WbTm[:, sl, :], WbTS[:, sl, :], dmb[:, 0:1])
            nc.vector.scalar_tensor_tensor(
                wm3[:, sl, :], deltaP[:, sl, :], dmb[:, 2:3], WbTm[:, sl, :],
                op0=ALU.mult, op1=ALU.add,
            )

        _mod_slice(slice(0, 3))
        nc.vector.tensor_add(comb8[:, 0, :], wm3[:, 1, :], wm3[:, 2, :])
        nc.vector.tensor_add(comb8[:, 1, :], wm3[:, 0, :], wm3[:, 1, :])
        _mod_slice(slice(6, 9))
        nc.vector.tensor_add(comb8[:, 6, :], wm3[:, 7, :], wm3[:, 8, :])
        nc.vector.tensor_add(comb8[:, 7, :], wm3[:, 6, :], wm3[:, 7, :])
        _mod_slice(slice(3, 6))
        nc.vector.tensor_add(R01[:], wm3[:, 3:6, :], wm3[:, 6:9, :])
        nc.vector.tensor_add(comb8[:, 2, :], R01[:, 1, :], R01[:, 2, :])
        nc.vector.tensor_add(comb8[:, 3, :], R01[:, 0, :], R01[:, 1, :])
        nc.vector.tensor_add(R10[:], wm3[:, 0:3, :], wm3[:, 3:6, :])
        nc.vector.tensor_add(comb8[:, 4, :], R10[:, 1, :], R10[:, 2, :])
        nc.vector.tensor_add(comb8[:, 5, :], R10[:, 0, :], R10[:, 1, :])
        rowsrc = {
            (0, 0): wm3[:, 0:3, :],
            (0, 1): R01[:],
            (1, 0): R10[:],
            (1, 1): wm3[:, 6:9, :],
        }

        # demod source: sq3 = wm3^2 (ACT), tap-sum on DVE.  The partition
        # sums (PE matmuls) are emitted inside the conv loop so the in-order
        # tensor queue doesn't stall the conv behind this reduce.
        sq3 = wtmp.tile([128, 9, C], f32)
        nc.scalar.square(sq3[:], wm3[:])
        nc.vector.tensor_reduce(
            s2[:], sq3.rearrange("p t o -> p o t"), axis=AX.X, op=ALU.add
        )

    def _conv_weight(di, dj, a, b):
        if dj == 0 and b == 0:
            return rowsrc[(di, a)][:, 0, :]
        if dj == 1 and b == 1:
            return rowsrc[(di, a)][:, 2, :]
        return comb8[:, 4 * di + 2 * a + dj, :]

    rowsrcF = {(1, 0): R10F[:], (1, 1): wm3F[:, 6:9, :]}

    def _conv_weightF(dj, a, b):  # di=1 only
        if dj == 0 and b == 0:
            return rowsrcF[(1, a)][:, 0, :]
        if dj == 1 and b == 1:
            return rowsrcF[(1, a)][:, 2, :]
        return comb4F[:, 2 * a + dj, :]

    def _pair_view(tt, col0):
        """[128, R, W, 2] view of a [128, R, WP] tile: (j, d) -> col j+d+col0."""
        ap = tt[:].copy()
        ap.ap = ap.ap[:-1] + [[1, W], [1, 2]]
        ap.offset = ap.offset + col0
        return ap

    # ---- main conv loop ----
    mpsum = ctx.enter_context(tc.tile_pool(name="mpsum", bufs=7, space="PSUM"))
    spsum = ctx.enter_context(tc.tile_pool(name="spsum", bufs=1, space="PSUM"))
    opool = ctx.enter_context(tc.tile_pool(name="obuf", bufs=3))
    upool = ctx.enter_context(tc.tile_pool(name="ubuf", bufs=3))

    # one PSUM bank shared by the three tiny demod tensors:
    # sP [128,1] | sProw [1,C] at col 4 | demB [128,C] at col 132
    dt_ = spsum.tile([128, 132 + C], f32)

    # Process the short tail block right after the KSWITCH warmup blocks so
    # its eviction+DMA latency hides mid-kernel instead of tailing the run.
    i0s = list(range(0, H, R_BLK))
    i0_order = i0s[:KSWITCH] + [i0s[-1]] + i0s[KSWITCH:-1]
    for bi, i0 in enumerate(i0_order):
        R = min(R_BLK, H - i0)
        bt, lo = _band_for(i0, R)
        # the final block uses the 16-matmul path: its eviction is one
        # engine-level deep, trimming the end-of-kernel latency tail that
        # the 3-op hybrid assembly chain would add
        hybrid = bi >= KSWITCH and R == R_BLK and bi != len(i0_order) - 1
        ob = opool.tile([128, R, 2, 2 * W], bf16, tag="ob", name=f"ob_{i0}")
        obv = ob.rearrange("p r d (j two) -> p r d two j", two=2)

        if not hybrid:
            ph = []
            for p in range(4):
                di, dj = p >> 1, p & 1
                pt = mpsum.tile([128, R * W], f32, tag="ph", name=f"ph{p}_{i0}")
                for q in range(4):
                    a, b = q >> 1, q & 1
                    r0 = i0 + (a + di - 1) - lo      # tile row of first x row
                    co = b + dj - 1
                    rhs = bt[:, r0 : r0 + R, co + 1 : co + 1 + W]
                    nc.tensor.matmul(
                        pt[:], _conv_weight(di, dj, a, b), rhs,
                        start=(q == 0), stop=(q == 3),
                    )
                ph.append(pt)
            if bi == 0:
                # demod partition sums, queued behind block 0's matmuls:
                # sP[o,1] for the eviction scale, sProw[1,o] for the fold.
                sP = dt_[:, 0:1]
                nc.tensor.matmul(sP, s2[:], onesS[:], start=True, stop=True)
                sProw = dt_[0:1, 4 : 4 + C]
                nc.tensor.matmul(sProw, onesS[:], s2[:], start=True, stop=True)
                t1 = const.tile([128, 1], f32)
                nc.vector.tensor_scalar_add(t1[:], sP, EPS)
                t2 = const.tile([128, 1], f32)
                nc.scalar.sqrt(t2[:], t1[:])
                nc.vector.reciprocal(demP[:], t2[:])
                r1 = const.tile([1, C], f32)
                nc.vector.tensor_scalar_add(r1[:], sProw, EPS)
                r2 = const.tile([1, C], f32)
                nc.scalar.sqrt(r2[:], r1[:])
                demRow = const.tile([1, C], f32)
                nc.vector.reciprocal(demRow[:], r2[:])
            if bi == 1:
                # demB[p, o] = dem[o] on every partition (outer product with
                # a ones row); by now demRow is ready so the PE doesn't stall.
                demB = dt_[:, 132 : 132 + C]
                nc.tensor.matmul(demB, onesRow[:], demRow[:], start=True, stop=True)
            # interleave phases into output rows; scale by demod, add bias
            last = bi == len(i0_order) - 1
            yv = y[:, 2 * i0 : 2 * i0 + 2 * R, :].rearrange(
                "p (r d) w -> p r d w", d=2
            )
            for p in range(4):
                di, dj = p >> 1, p & 1
                dst = obv[:, :, di, dj, :]
                srcv = ph[p].rearrange("p (r j) -> p r j", r=R)
                if dj == 0:
                    nc.vector.tensor_scalar(
                        dst, srcv, demP[:, 0:1], dmb[:, 1:2],
                        op0=ALU.mult, op1=ALU.add,
                    )
                else:
                    nc.scalar.activation(
                        dst, srcv, AF.Identity, bias=dmb[:, 1:2], scale=demP[:, 0:1]
                    )
                if last and p == 1:
                    # drain even output rows while odd rows still evict
                    nc.sync.dma_start(yv[:, :, 0, :], ob[:, :, 0, :])
            if last:
                nc.sync.dma_start(yv[:, :, 1, :], ob[:, :, 1, :])
            if bi == 1:
                # fold demod into the weights for the hybrid blocks
                demB = dt_[:, 132 : 132 + C]
                demBt = demB.unsqueeze(1).broadcast_to([128, 9, C])
                nc.vector.tensor_tensor(wm3F[:], wm3[:], demBt, op=ALU.mult)
                nc.vector.tensor_add(R01F[:], wm3F[:, 3:6, :], wm3F[:, 6:9, :])
                nc.vector.tensor_add(R10F[:], wm3F[:, 0:3, :], wm3F[:, 3:6, :])
                for a in range(2):
                    S = rowsrcF[(1, a)]
                    nc.vector.tensor_add(
                        comb4F[:, 2 * a, :], S[:, 1, :], S[:, 2, :]
                    )
                    nc.vector.tensor_add(
                        comb4F[:, 2 * a + 1, :], S[:, 0, :], S[:, 1, :]
                    )
        else:
            # ---- hybrid 14-matmul block ----
            # di=0: three column-tap tiles t[kj], 2 row-shift matmuls each
            tts = []
            for kj in range(3):
                tt = mpsum.tile([128, R, WP], f32, tag="ph", name=f"tt{kj}_{i0}")
                for a in range(2):
                    wgt = wm3F[:, kj, :] if a == 0 else R01F[:, kj, :]
                    r0 = i0 + a - 1 - lo
                    nc.tensor.matmul(
                        tt[:], wgt, bt[:, r0 : r0 + R, :],
                        start=(a == 0), stop=(a == 1),
                    )
                tts.append(tt)
            # di=1: direct 4-tap phases
            pts = []
            for dj in range(2):
                pt = mpsum.tile([128, R * W], f32, tag="ph", name=f"q{dj}_{i0}")
                for q in range(4):
                    a, b = q >> 1, q & 1
                    r0 = i0 + a - lo
                    co = b + dj - 1
                    rhs = bt[:, r0 : r0 + R, co + 1 : co + 1 + W]
                    nc.tensor.matmul(
                        pt[:], _conv_weightF(dj, a, b), rhs,
                        start=(q == 0), stop=(q == 3),
                    )
                pts.append(pt)
            # di=0 assembly:
            #   y[2i, 2j+d] = t0[j+d] + t1[j+1] + t2[j+1+d] + bias
            # Each op reads at most one PSUM operand (DVE single-PSUM-port):
            #   ACT: sA[j]     = t1[j+1] + bias
            #   DVE: u[j,d]    = t2[j+1+d] + sA[j]     (0-stride pair bcast)
            #   DVE: ob0[j,d]  = t0[j+d] + u[j,d]
            sA = upool.tile([128, R, W], bf16, tag="sA", name=f"sA_{i0}")
            nc.scalar.activation(
                sA[:], tts[1][:, :, 1 : 1 + W], AF.Identity, bias=dmb[:, 1:2]
            )
            u = upool.tile([128, R, 2 * W], bf16, tag="u", name=f"u_{i0}")
            uv = u.rearrange("p r (j d) -> p r j d", d=2)
            sA4 = sA[:].unsqueeze(-1).broadcast_to([128, R, W, 2])
            nc.vector.tensor_tensor(uv, _pair_view(tts[2], 1), sA4, op=ALU.add)
            ob0v = ob[:, :, 0, :].rearrange("p r (j d) -> p r j d", d=2)
            nc.vector.tensor_tensor(ob0v, _pair_view(tts[0], 0), uv, op=ALU.add)
            # di=1 evictions on ACT, bias only (demod already in weights)
            for dj in range(2):
                dst = obv[:, :, 1, dj, :]
                srcv = pts[dj].rearrange("p (r j) -> p r j", r=R)
                nc.scalar.activation(dst, srcv, AF.Identity, bias=dmb[:, 1:2])

        nc.sync.dma_start(y[:, 2 * i0 : 2 * i0 + 2 * R, :], ob[:])


def _build():
    nc = bacc.Bacc(
        "TRN2",
        target_bir_lowering=False,
        debug=False,
        enable_asserts=False,
        num_devices=NCORES,
    )
    x = nc.dram_tensor("x", [C, H + 2, W + 2], bf16, kind="ExternalInput").ap()
    dmbias = nc.dram_tensor("dmbias", [2, C], f32, kind="ExternalInput").ap()
    wbT = nc.dram_tensor("WbT", [C, 9 * C], bf16, kind="ExternalInput").ap()
    luT = nc.dram_tensor("lora_upT", [RANK, C], bf16, kind="ExternalInput").ap()
    ldT = nc.dram_tensor("lora_downT", [RANK, 9 * C], bf16, kind="ExternalInput").ap()
    ident2 = nc.dram_tensor("ident2", [2, 2], f32, kind="ExternalInput").ap()
    y = nc.dram_tensor("y", [C, 2 * H, 2 * W], bf16, kind="ExternalOutput").ap()

    with tile.TileContext(nc) as tc:
        with ExitStack() as ctx:
            _conv_kernel(ctx, tc, y, x, dmbias, wbT, luT, ldT, ident2)
    nc.compile()
    return nc


_CACHE = {}


def _get_nc():
    if "nc" not in _CACHE:
        _CACHE["nc"] = _build()
    return _CACHE["nc"]


def _make_in_maps(x, de_mod, Wb, lora_up, lora_down, bias):
    bf = ml_dtypes.bfloat16
    x = np.asarray(x, dtype=np.float32).astype(bf)
    # zero-pad the spatial borders on the host: the band DMAs then deliver
    # halo rows/columns directly (layout-only prep)
    xp = np.zeros((B, C, H + 2, W + 2), dtype=bf)
    xp[:, :, 1 : H + 1, 1 : W + 1] = x
    de_mod = np.asarray(de_mod, dtype=np.float32)
    Wb = np.asarray(Wb, dtype=np.float32)
    lora_up = np.asarray(lora_up, dtype=np.float32)
    lora_down = np.asarray(lora_down, dtype=np.float32)
    # layout-only host prep: [O,I,3,3] -> [i, (t o)], [R,C,3,3] -> [r, (t i)]
    wbT = np.ascontiguousarray(Wb.transpose(1, 2, 3, 0).reshape(C, 9 * C)).astype(bf)
    luT = np.ascontiguousarray(lora_up.T).astype(bf)
    ldT = np.ascontiguousarray(
        lora_down.transpose(0, 2, 3, 1).reshape(RANK, 9 * C)
    ).astype(bf)
    bias = np.asarray(bias, dtype=np.float32).reshape(C)
    id2 = np.eye(2, dtype=np.float32)
    in_maps = []
    for b in range(NCORES):
        in_maps.append(
            {
                "x": np.ascontiguousarray(xp[b]),
                "dmbias": np.ascontiguousarray(np.stack([de_mod[b], bias])),
                "WbT": wbT,
                "lora_upT": luT,
                "lora_downT": ldT,
                "ident2": id2,
            }
        )
    return in_maps


def run(inputs, trace=False, trace_kwargs=None):
    nc = _get_nc()
    in_maps = _make_in_maps(**inputs)
    res = run_bass_kernel_spmd(
        nc,
        in_maps,
        core_ids=list(range(NCORES)),
        trace=trace,
        **(trace_kwargs or {}),
    )
    y = np.stack(
        [res.results[b]["y"].astype(np.float32) for b in range(NCORES)], axis=0
    )
    return y, res


def kernel(**inputs):
    y, _ = run(inputs)
    return y


# revision 33
# speedup vs baseline: 1.1941x; 1.0250x over previous
"""Trainium2 Bass kernel for nn_NeuronS3DiffUpsample2D.

Reference computation (per sample b):
    up   = nearest-2x-upsample(x[b])                       # [C, 320, 320]
    w    = Wb + 0.25 * einsum('or,rikl->oikl', lora_up, lora_down)
    w_b  = w * de_mod[b, None, :, None, None]              # modulate input chans
    dem  = rsqrt(sum_{i,k,l} w_b^2 + eps)                  # per output chan
    y[b] = conv2d(up, w_b * dem, SAME) + bias

Key algebraic transform: a 3x3 SAME conv on a 2x nearest-upsampled image
decomposes into 4 output phases (di, dj in {0,1}), each a 2x2 conv on the
ORIGINAL 160x160 input:
    y[2i+di, 2j+dj] = sum_{a,b in {0,1}} K[di,dj,a,b] @ x[i+a+di-1, j+b+dj-1]
where the 16 [O, I] matrices K are sums of 1/2/4 of the 9 taps of w.
This is 4/9 of the naive FLOPs and never materializes the upsampled image.

On top of that, the steady-state loop uses a 14-matmul block: the two di=0
phases share three column-tap tiles t[kj][i,j] = sum_a w[S(0,a),kj] (x) at
row shifts (6 matmuls, PE) and are assembled by the DVE as
    y[2i, 2j+dj] = t0[j+dj] + t1[j+1] + t2[j+1+dj] + bias
(shifted overlapping-pair access patterns; t1 uses a 0-stride broadcast
pair).  The di=1 phases stay as direct 4-tap accumulations (8 matmuls)
evicted by the ACT engine.  This trades 2 of 16 PE matmuls per block for
DVE/ACT work that fits in their idle capacity: PE is the only saturated
engine (~90% busy at 1 col/cycle).

For the assembly to be a pure add, the demod scale is folded INTO the
weights.  demod depends on the weights themselves, so the first KSWITCH
blocks run the original 16-matmul path (demod applied per-partition at PSUM
eviction) while the fold chain (row-form demod via a [1,C] PE reduction, a
broadcast outer product, and a DVE multiply) completes in the background.

Sharding: data-parallel over batch B=8 across 8 NeuronCores; each core builds
its own per-sample weights locally.  Host-side work is layout only (slicing,
transposition, fp32->bf16 rounding).  All arithmetic is on device.

Everything runs in bf16 (PE streams bf16 at 1 cycle/row like f32r, but
LDWEIGHTS fully hides under the previous 480-col stream, and input/output
DMA halves; measured rel err 4e-3 vs the 2e-2 budget).  Output goes to DRAM
as bf16 and is widened to f32 on the host.

The input arrives host-padded ([C, 162, 162] with zero borders) so a single
contiguous DMA per band delivers data and halos together — on-device border
zero-writes race with the unaligned bf16 DMA's write granules (observed as
nondeterministic right-edge outputs).  The first band is short so the first
conv matmul isn't gated behind a 1.2MB transfer, and the tail block is
processed early so its eviction+DMA latency hides mid-kernel.  Weight DMAs
are split across the sync and ACT queues; bands ride the gpsimd SWDGE queue.

Measured on 8 axon-tunneled TRN2 cores: 186.0us (vs 225.6us baseline) at
the PE stream roofline (204.5ns per 480-col bf16 matmul, Tensor ~90% busy);
run-to-run device clock variance is ~±10%.
"""

import sys
import numpy as np
import ml_dtypes
from contextlib import ExitStack

try:
    import concourse.bass as bass
except ImportError:  # grading env without the axon PYTHONPATH
    sys.path.insert(0, "/opt/trn_rl_repo")
    import concourse.bass as bass
import concourse.tile as tile
from concourse import bacc, mybir
from concourse.bass_utils import run_bass_kernel_spmd

B, C, H, W = 8, 128, 160, 160
RANK = 32
SCALING = 0.25
EPS = 1e-8
WP = W + 2          # padded row length (zero col on each side)
R_BLK = 3           # x-rows per matmul block -> N = 3*160 = 480 <= 512
# band cut points: short first band so block 0 starts early
CUTS = [0, 9, 36, 63, 90, 117, 144, 160]
KSWITCH = 4         # blocks on the 16-matmul path while demod-fold completes
NCORES = 8

f32 = mybir.dt.float32
bf16 = mybir.dt.bfloat16


def _conv_kernel(ctx, tc, y, x, dmbias, wbT, luT, ldT, ident2):
    nc = tc.nc
    AF = mybir.ActivationFunctionType
    ALU = mybir.AluOpType
    AX = mybir.AxisListType

    const = ctx.enter_context(tc.tile_pool(name="const", bufs=1))
    bands = ctx.enter_context(tc.tile_pool(name="bands", bufs=7))

    # dmbias/ident2 (tiny, gate the de_mod transpose) on sync; weight
    # tensors on the otherwise-idle ACT queue; bands on gpsimd SWDGE.
    dmbR = const.tile([2, C], f32)
    nc.sync.dma_start(dmbR[:], dmbias[:])
    id2 = const.tile([2, 2], f32)
    nc.sync.dma_start(id2[:], ident2[:])
    # WbT split across the sync and ACT queues so the two halves transfer
    # in parallel (a single 295KB DMA gated the whole weight chain)
    # WbT split in three so the chunks transfer in parallel on the sync,
    # ACT and gpsimd queues (a single 295KB DMA gated the weight chain)
    WbTS = const.tile([128, 9, C], bf16)         # Wb^T: [i, t, o]
    nc.sync.dma_start(WbTS[:, 0:5, :], wbT[:, 0 : 5 * C])
    LD9 = const.tile([RANK, 9, C], bf16)         # lora_down^T: [r, t, i]
    nc.scalar.dma_start(LD9[:], ldT[:])
    LUTn = const.tile([RANK, C], bf16)           # lora_up^T: [r, o]
    nc.scalar.dma_start(LUTn[:], luT[:])
    nc.scalar.dma_start(WbTS[:, 5:9, :], wbT[:, 5 * C : 9 * C])

    # weight tensors the conv loop reads as stationary operands
    wm3 = const.tile([128, 9, C], bf16)          # modulated w^T: [i, t, o]
    R01 = const.tile([128, 3, C], bf16)          # rows ki1+ki2
    R10 = const.tile([128, 3, C], bf16)          # rows ki0+ki1
    comb8 = const.tile([128, 8, C], bf16)        # two-column tap sums (unfolded)
    wm3F = const.tile([128, 9, C], bf16)         # demod-folded variants
    R01F = const.tile([128, 3, C], bf16)
    R10F = const.tile([128, 3, C], bf16)
    comb4F = const.tile([128, 4, C], bf16)       # folded di=1 two-column sums
    demP = const.tile([128, 1], f32)             # rsqrt demod, per output chan
    dmb = const.tile([128, 3], f32)              # de_mod[i], bias[o], 0.25*de_mod
    s2 = const.tile([128, C], f32)               # per-(i,o) tap-summed squares
    onesS = const.tile([128, 1], f32)
    onesRow = const.tile([1, C], f32)

    nc.vector.memset(onesS[:], 1.0)
    nc.vector.memset(onesRow[:], 1.0)

    # ---- input bands: (lo, hi) are halo-inclusive x-row bounds.  x arrives
    # pre-padded on the host ([C, H+2, W+2] with zero borders), so a single
    # whole-tile contiguous DMA delivers data AND halos: no on-device border
    # writes (a disjoint border zero-write races with the DMA's write
    # granules on HW), and per-partition descriptors coalesce.
    segs = [(CUTS[i] - 1, min(CUTS[i + 1], H)) for i in range(len(CUTS) - 1)]
    # DMA issue order: bands 0 and 1 first (consumed earliest), then the
    # LAST band (the tail block is processed early, see below), then the
    # rest in order.
    dma_order = [0, 1, len(segs) - 1] + list(range(2, len(segs) - 1))
    band_tiles = [None] * len(segs)
    for si in dma_order:
        lo, hi = segs[si]
        nrows = hi - lo + 1
        bt = bands.tile([128, nrows, WP], bf16, tag="band", name=f"band{lo}")
        nc.gpsimd.dma_start(bt[:], x[:, lo + 1 : hi + 2, :])
        band_tiles[si] = (bt, lo, hi)

    def _band_for(i0, R):
        for bt, lo, hi in band_tiles:
            if lo <= i0 - 1 and i0 + R <= hi:
                return bt, lo
        raise AssertionError(f"no band for block {i0}")

    # ---- weight stage ----
    with tc.tile_pool(name="wtmp", bufs=1) as wtmp, tc.tile_pool(
        name="wpsum", bufs=1, space="PSUM"
    ) as wpsum:
        dmbP = wpsum.tile([128, 2], f32)
        nc.tensor.transpose(dmbP[:], dmbR[:], id2[:])
        nc.vector.tensor_copy(dmb[:, 0:2], dmbP[:])
        nc.vector.tensor_scalar_mul(dmb[:, 2:3], dmb[:, 0:1], SCALING)

        # deltaT_unscaled[i, t, o] = sum_r down[r,i,t] * up[o,r]; the 0.25
        # lora scale rides in via the fused modulation below
        deltaP = wpsum.tile([128, 9, C], f32)
        for t in range(9):
            nc.tensor.matmul(
                deltaP[:, t, :], LD9[:, t, :], LUTn[:], start=True, stop=True
            )

        # wm3 = Wb^T*dm + deltaT*(0.25*dm), in three tap-slices matching the
        # three WbT DMA chunks so the modulation chain starts on the first
        # bytes to land; the slices the first conv phase reads come first.
        # Row combos over ki (t = 3*ki + kj):
        #   (di=0, a=0): ki0        (di=0, a=1): ki1+ki2
        #   (di=1, a=0): ki0+ki1    (di=1, a=1): ki2
        # Single-column taps are read directly out of wm3/R01/R10; only the
        # two-column sums are materialized (slot = 4*di + 2*a + dj).
        WbTm = wtmp.tile([128, 9, C], bf16)

        def _mod_slice(sl):
            nc.vector.tensor_scalar_mul(WbTm[:, sl, :], WbTS[:, sl, :], dmb[:, 0:1])
            nc.vector.scalar_tensor_tensor(
                wm3[:, sl, :], deltaP[:, sl, :], dmb[:, 2:3], WbTm[:, sl, :],
                op0=ALU.mult, op1=ALU.add,
            )

        _mod_slice(slice(0, 3))
        nc.vector.tensor_add(comb8[:, 0, :], wm3[:, 1, :], wm3[:, 2, :])
        nc.vector.tensor_add(comb8[:, 1, :], wm3[:, 0, :], wm3[:, 1, :])
        _mod_slice(slice(6, 9))
        nc.vector.tensor_add(comb8[:, 6, :], wm3[:, 7, :], wm3[:, 8, :])
        nc.vector.tensor_add(comb8[:, 7, :], wm3[:, 6, :], wm3[:, 7, :])
        _mod_slice(slice(3, 6))
        nc.vector.tensor_add(R01[:], wm3[:, 3:6, :], wm3[:, 6:9, :])
        nc.vector.tensor_add(comb8[:, 2, :], R01[:, 1, :], R01[:, 2, :])
        nc.vector.tensor_add(comb8[:, 3, :], R01[:, 0, :], R01[:, 1, :])
        nc.vector.tensor_add(R10[:], wm3[:, 0:3, :], wm3[:, 3:6, :])
        nc.vector.tensor_add(comb8[:, 4, :], R10[:, 1, :], R10[:, 2, :])
        nc.vector.tensor_add(comb8[:, 5, :], R10[:, 0, :], R10[:, 1, :])
        rowsrc = {
            (0, 0): wm3[:, 0:3, :],
            (0, 1): R01[:],
            (1, 0): R10[:],
            (1, 1): wm3[:, 6:9, :],
        }

        # demod source: sq3 = wm3^2 (ACT), tap-sum on DVE.  The partition
        # sums (PE matmuls) are emitted inside the conv loop so the in-order
        # tensor queue doesn't stall the conv behind this reduce.
        sq3 = wtmp.tile([128, 9, C], f32)
        nc.scalar.square(sq3[:], wm3[:])
        nc.vector.tensor_reduce(
            s2[:], sq3.rearrange("p t o -> p o t"), axis=AX.X, op=ALU.add
        )

    def _conv_weight(di, dj, a, b):
        if dj == 0 and b == 0:
            return rowsrc[(di, a)][:, 0, :]
        if dj == 1 and b == 1:
            return rowsrc[(di, a)][:, 2, :]
        return comb8[:, 4 * di + 2 * a + dj, :]

    rowsrcF = {(1, 0): R10F[:], (1, 1): wm3F[:, 6:9, :]}

    def _conv_weightF(dj, a, b):  # di=1 only
        if dj == 0 and b == 0:
            return rowsrcF[(1, a)][:, 0, :]
        if dj == 1 and b == 1:
            return rowsrcF[(1, a)][:, 2, :]
        return comb4F[:, 2 * a + dj, :]

    def _pair_view(tt, col0):
        """[128, R, W, 2] view of a [128, R, WP] tile: (j, d) -> col j+d+col0."""
        ap = tt[:].copy()
        ap.ap = ap.ap[:-1] + [[1, W], [1, 2]]
        ap.offset = ap.offset + col0
        return ap

    # ---- main conv loop ----
    mpsum = ctx.enter_context(tc.tile_pool(name="mpsum", bufs=7, space="PSUM"))
    spsum = ctx.enter_context(tc.tile_pool(name="spsum", bufs=1, space="PSUM"))
    opool = ctx.enter_context(tc.tile_pool(name="obuf", bufs=3))
    upool = ctx.enter_context(tc.tile_pool(name="ubuf", bufs=3))

    # one PSUM bank shared by the three tiny demod tensors:
    # sP [128,1] | sProw [1,C] at col 4 | demB [128,C] at col 132
    dt_ = spsum.tile([128, 132 + C], f32)

    # Process the short tail block right after the KSWITCH warmup blocks so
    # its eviction+DMA latency hides mid-kernel instead of tailing the run.
    i0s = list(range(0, H, R_BLK))
    i0_order = i0s[:KSWITCH] + [i0s[-1]] + i0s[KSWITCH:-1]
    for bi, i0 in enumerate(i0_order):
        R = min(R_BLK, H - i0)
        bt, lo = _band_for(i0, R)
        # the final block uses the 16-matmul path: its eviction is one
        # engine-level deep, trimming the end-of-kernel latency tail that
        # the 3-op hybrid assembly chain would add
        hybrid = bi >= KSWITCH and R == R_BLK and bi != len(i0_order) - 1
        ob = opool.tile([128, R, 2, 2 * W], bf16, tag="ob", name=f"ob_{i0}")
        obv = ob.rearrange("p r d (j two) -> p r d two j", two=2)

        if not hybrid:
            ph = []
            for p in range(4):
                di, dj = p >> 1, p & 1
                pt = mpsum.tile([128, R * W], f32, tag="ph", name=f"ph{p}_{i0}")
                for q in range(4):
                    a, b = q >> 1, q & 1
                    r0 = i0 + (a + di - 1) - lo      # tile row of first x row
                    co = b + dj - 1
                    rhs = bt[:, r0 : r0 + R, co + 1 : co + 1 + W]
                    nc.tensor.matmul(
                        pt[:], _conv_weight(di, dj, a, b), rhs,
                        start=(q == 0), stop=(q == 3),
                    )
                ph.append(pt)
            if bi == 0:
                # demod partition sums, queued behind block 0's matmuls:
                # sP[o,1] for the eviction scale, sProw[1,o] for the fold.
                sP = dt_[:, 0:1]
                nc.tensor.matmul(sP, s2[:], onesS[:], start=True, stop=True)
                sProw = dt_[0:1, 4 : 4 + C]
                nc.tensor.matmul(sProw, onesS[:], s2[:], start=True, stop=True)
                t1 = const.tile([128, 1], f32)
                nc.vector.tensor_scalar_add(t1[:], sP, EPS)
                t2 = const.tile([128, 1], f32)
                nc.scalar.sqrt(t2[:], t1[:])
                nc.vector.reciprocal(demP[:], t2[:])
                r1 = const.tile([1, C], f32)
                nc.vector.tensor_scalar_add(r1[:], sProw, EPS)
                r2 = const.tile([1, C], f32)
                nc.scalar.sqrt(r2[:], r1[:])
                demRow = const.tile([1, C], f32)
                nc.vector.reciprocal(demRow[:], r2[:])
            if bi == 1:
                # demB[p, o] = dem[o] on every partition (outer product with
                # a ones row); by now demRow is ready so the PE doesn't stall.
                demB = dt_[:, 132 : 132 + C]
                nc.tensor.matmul(demB, onesRow[:], demRow[:], start=True, stop=True)
            # interleave phases into output rows; scale by demod, add bias
            for p in range(4):
                di, dj = p >> 1, p & 1
                dst = obv[:, :, di, dj, :]
                srcv = ph[p].rearrange("p (r j) -> p r j", r=R)
                if dj == 0:
                    nc.vector.tensor_scalar(
                        dst, srcv, demP[:, 0:1], dmb[:, 1:2],
                        op0=ALU.mult, op1=ALU.add,
                    )
                else:
                    nc.scalar.activation(
                        dst, srcv, AF.Identity, bias=dmb[:, 1:2], scale=demP[:, 0:1]
                    )
            if bi == 1:
                # fold demod into the weights for the hybrid blocks
                demB = dt_[:, 132 : 132 + C]
                demBt = demB.unsqueeze(1).broadcast_to([128, 9, C])
                nc.vector.tensor_tensor(wm3F[:], wm3[:], demBt, op=ALU.mult)
                nc.vector.tensor_add(R01F[:], wm3F[:, 3:6, :], wm3F[:, 6:9, :])
                nc.vector.tensor_add(R10F[:], wm3F[:, 0:3, :], wm3F[:, 3:6, :])
                for a in range(2):
                    S = rowsrcF[(1, a)]
                    nc.vector.tensor_add(
                        comb4F[:, 2 * a, :], S[:, 1, :], S[:, 2, :]
                    )
                    nc.vector.tensor_add(
                        comb4F[:, 2 * a + 1, :], S[:, 0, :], S[:, 1, :]
                    )
        else:
            # ---- hybrid 14-matmul block ----
            # di=0: three column-tap tiles t[kj], 2 row-shift matmuls each
            tts = []
            for kj in range(3):
                tt = mpsum.tile([128, R, WP], f32, tag="ph", name=f"tt{kj}_{i0}")
                for a in range(2):
                    wgt = wm3F[:, kj, :] if a == 0 else R01F[:, kj, :]
                    r0 = i0 + a - 1 - lo
                    nc.tensor.matmul(
                        tt[:], wgt, bt[:, r0 : r0 + R, :],
                        start=(a == 0), stop=(a == 1),
                    )
                tts.append(tt)
            # di=1: direct 4-tap phases
            pts = []
            for dj in range(2):
                pt = mpsum.tile([128, R * W], f32, tag="ph", name=f"q{dj}_{i0}")
                for q in range(4):
                    a, b = q >> 1, q & 1
                    r0 = i0 + a - lo
                    co = b + dj - 1
                    rhs = bt[:, r0 : r0 + R, co + 1 : co + 1 + W]
                    nc.tensor.matmul(
                        pt[:], _conv_weightF(dj, a, b), rhs,
                        start=(q == 0), stop=(q == 3),
                    )
                pts.append(pt)
            # di=0 assembly:
            #   y[2i, 2j+d] = t0[j+d] + t1[j+1] + t2[j+1+d] + bias
            # Each op reads at most one PSUM operand (DVE single-PSUM-port):
            #   ACT: sA[j]     = t1[j+1] + bias
            #   DVE: u[j,d]    = t2[j+1+d] + sA[j]     (0-stride pair bcast)
            #   DVE: ob0[j,d]  = t0[j+d] + u[j,d]
            sA = upool.tile([128, R, W], bf16, tag="sA", name=f"sA_{i0}")
            nc.scalar.activation(
                sA[:], tts[1][:, :, 1 : 1 + W], AF.Identity, bias=dmb[:, 1:2]
            )
            u = upool.tile([128, R, 2 * W], bf16, tag="u", name=f"u_{i0}")
            uv = u.rearrange("p r (j d) -> p r j d", d=2)
            sA4 = sA[:].unsqueeze(-1).broadcast_to([128, R, W, 2])
            nc.vector.tensor_tensor(uv, _pair_view(tts[2], 1), sA4, op=ALU.add)
            ob0v = ob[:, :, 0, :].rearrange("p r (j d) -> p r j d", d=2)
            nc.vector.tensor_tensor(ob0v, _pair_view(tts[0], 0), uv, op=ALU.add)
            # di=1 evictions on ACT, bias only (demod already in weights)
            for dj in range(2):
                dst = obv[:, :, 1, dj, :]
                srcv = pts[dj].rearrange("p (r j) -> p r j", r=R)
                nc.scalar.activation(dst, srcv, AF.Identity, bias=dmb[:, 1:2])

        nc.sync.dma_start(y[:, 2 * i0 : 2 * i0 + 2 * R, :], ob[:])


def _build():
    nc = bacc.Bacc(
        "TRN2",
        target_bir_lowering=False,
        debug=False,
        enable_asserts=False,
        num_devices=NCORES,
    )
    x = nc.dram_tensor("x", [C, H + 2, W + 2], bf16, kind="ExternalInput").ap()
    dmbias = nc.dram_tensor("dmbias", [2, C], f32, kind="ExternalInput").ap()
    wbT = nc.dram_tensor("WbT", [C, 9 * C], bf16, kind="ExternalInput").ap()
    luT = nc.dram_tensor("lora_upT", [RANK, C], bf16, kind="ExternalInput").ap()
    ldT = nc.dram_tensor("lora_downT", [RANK, 9 * C], bf16, kind="ExternalInput").ap()
    ident2 = nc.dram_tensor("ident2", [2, 2], f32, kind="ExternalInput").ap()
    y = nc.dram_tensor("y", [C, 2 * H, 2 * W], bf16, kind="ExternalOutput").ap()

    with tile.TileContext(nc) as tc:
        with ExitStack() as ctx:
            _conv_kernel(ctx, tc, y, x, dmbias, wbT, luT, ldT, ident2)
    nc.compile()
    return nc


_CACHE = {}


def _get_nc():
    if "nc" not in _CACHE:
        _CACHE["nc"] = _build()
    return _CACHE["nc"]


def _make_in_maps(x, de_mod, Wb, lora_up, lora_down, bias):
    bf = ml_dtypes.bfloat16
    x = np.asarray(x, dtype=np.float32).astype(bf)
    # zero-pad the spatial borders on the host: the band DMAs then deliver
    # halo rows/columns directly (layout-only prep)
    xp = np.zeros((B, C, H + 2, W + 2), dtype=bf)
    xp[:, :, 1 : H + 1, 1 : W + 1] = x
    de_mod = np.asarray(de_mod, dtype=np.float32)
    Wb = np.asarray(Wb, dtype=np.float32)
    lora_up = np.asarray(lora_up, dtype=np.float32)
    lora_down = np.asarray(lora_down, dtype=np.float32)
    # layout-only host prep: [O,I,3,3] -> [i, (t o)], [R,C,3,3] -> [r, (t i)]
    wbT = np.ascontiguousarray(Wb.transpose(1, 2, 3, 0).reshape(C, 9 * C)).astype(bf)
    luT = np.ascontiguousarray(lora_up.T).astype(bf)
    ldT = np.ascontiguousarray(
        lora_down.transpose(0, 2, 3, 1).reshape(RANK, 9 * C)
    ).astype(bf)
    bias = np.asarray(bias, dtype=np.float32).reshape(C)
    id2 = np.eye(2, dtype=np.float32)
    in_maps = []
    for b in range(NCORES):
        in_maps.append(
            {
                "x": np.ascontiguousarray(xp[b]),
                "dmbias": np.ascontiguousarray(np.stack([de_mod[b], bias])),
                "WbT": wbT,
                "lora_upT": luT,
                "lora_downT": ldT,
                "ident2": id2,
            }
        )
    return in_maps


def run(inputs, trace=False, trace_kwargs=None):
    nc = _get_nc()
    in_maps = _make_in_maps(**inputs)
    res = run_bass_kernel_spmd(
        nc,
        in_maps,
        core_ids=list(range(NCORES)),
        trace=trace,
        **(trace_kwargs or {}),
    )
    y = np.stack(
        [res.results[b]["y"].astype(np.float32) for b in range(NCORES)], axis=0
    )
    return y, res


def kernel(**inputs):
    y, _ = run(inputs)
    return y


# revision 36
# speedup vs baseline: 1.2000x; 1.0049x over previous
"""Trainium2 Bass kernel for nn_NeuronS3DiffUpsample2D.

Reference computation (per sample b):
    up   = nearest-2x-upsample(x[b])                       # [C, 320, 320]
    w    = Wb + 0.25 * einsum('or,rikl->oikl', lora_up, lora_down)
    w_b  = w * de_mod[b, None, :, None, None]              # modulate input chans
    dem  = rsqrt(sum_{i,k,l} w_b^2 + eps)                  # per output chan
    y[b] = conv2d(up, w_b * dem, SAME) + bias

Key algebraic transform: a 3x3 SAME conv on a 2x nearest-upsampled image
decomposes into 4 output phases (di, dj in {0,1}), each a 2x2 conv on the
ORIGINAL 160x160 input:
    y[2i+di, 2j+dj] = sum_{a,b in {0,1}} K[di,dj,a,b] @ x[i+a+di-1, j+b+dj-1]
where the 16 [O, I] matrices K are sums of 1/2/4 of the 9 taps of w.
This is 4/9 of the naive FLOPs and never materializes the upsampled image.

On top of that, the steady-state loop uses a 14-matmul block: the two di=0
phases share three column-tap tiles t[kj][i,j] = sum_a w[S(0,a),kj] (x) at
row shifts (6 matmuls, PE) and are assembled by the DVE as
    y[2i, 2j+dj] = t0[j+dj] + t1[j+1] + t2[j+1+dj] + bias
(shifted overlapping-pair access patterns; t1 uses a 0-stride broadcast
pair).  The di=1 phases stay as direct 4-tap accumulations (8 matmuls)
evicted by the ACT engine.  This trades 2 of 16 PE matmuls per block for
DVE/ACT work that fits in their idle capacity: PE is the only saturated
engine (~90% busy at 1 col/cycle).

For the assembly to be a pure add, the demod scale is folded INTO the
weights.  demod depends on the weights themselves, so the first KSWITCH
blocks run the original 16-matmul path (demod applied per-partition at PSUM
eviction) while the fold chain (row-form demod via a [1,C] PE reduction, a
broadcast outer product, and a DVE multiply) completes in the background.

Sharding: data-parallel over batch B=8 across 8 NeuronCores; each core builds
its own per-sample weights locally.  Host-side work is layout only (slicing,
transposition, fp32->bf16 rounding).  All arithmetic is on device.

Everything runs in bf16 (PE streams bf16 at 1 cycle/row like f32r, but
LDWEIGHTS fully hides under the previous 480-col stream, and input/output
DMA halves; measured rel err 4e-3 vs the 2e-2 budget).  Output goes to DRAM
as bf16 and is widened to f32 on the host.

The input arrives host-padded ([C, 162, 162] with zero borders) so a single
contiguous DMA per band delivers data and halos together — on-device border
zero-writes race with the unaligned bf16 DMA's write granules (observed as
nondeterministic right-edge outputs).  The first band is short so the first
conv matmul isn't gated behind a 1.2MB transfer, and the tail block is
processed early so its eviction+DMA latency hides mid-kernel.  Weight DMAs
are split across the sync and ACT queues; bands ride the gpsimd SWDGE queue.

Measured on 8 axon-tunneled TRN2 cores: 185.0us (vs 225.6us baseline) at
the PE stream roofline (204.5ns per 480-col bf16 matmul, Tensor ~88% busy,
DVE 72%, ACT 68%); run-to-run device clock variance is ~±10%.
"""

import sys
import numpy as np
import ml_dtypes
from contextlib import ExitStack

try:
    import concourse.bass as bass
except ImportError:  # grading env without the axon PYTHONPATH
    sys.path.insert(0, "/opt/trn_rl_repo")
    import concourse.bass as bass
import concourse.tile as tile
from concourse import bacc, mybir
from concourse.bass_utils import run_bass_kernel_spmd

B, C, H, W = 8, 128, 160, 160
RANK = 32
SCALING = 0.25
EPS = 1e-8
WP = W + 2          # padded row length (zero col on each side)
R_BLK = 3           # x-rows per matmul block -> N = 3*160 = 480 <= 512
# band cut points: short first band so block 0 starts early
CUTS = [0, 9, 36, 63, 90, 117, 144, 160]
KSWITCH = 4         # blocks on the 16-matmul path while demod-fold completes
NCORES = 8

f32 = mybir.dt.float32
bf16 = mybir.dt.bfloat16


def _conv_kernel(ctx, tc, y, x, dmbias, wbT, luT, ldT, ident2):
    nc = tc.nc
    AF = mybir.ActivationFunctionType
    ALU = mybir.AluOpType
    AX = mybir.AxisListType

    const = ctx.enter_context(tc.tile_pool(name="const", bufs=1))
    bands = ctx.enter_context(tc.tile_pool(name="bands", bufs=7))

    # dmbias/ident2 (tiny, gate the de_mod transpose) on sync; weight
    # tensors on the otherwise-idle ACT queue; bands on gpsimd SWDGE.
    # The early-queue DMA bandwidth is low (~35GB/s while ramping), so the
    # weight tensors are spread across FOUR queues (sync, vector, ACT;
    # gpsimd carries the bands) to minimize the latest arrival, which gates
    # the whole weight chain and thus the first conv matmul.
    dmbR = const.tile([2, C], f32)
    nc.sync.dma_start(dmbR[:], dmbias[:])
    id2 = const.tile([2, 2], f32)
    nc.sync.dma_start(id2[:], ident2[:])
    WbTS = const.tile([128, 9, C], bf16)         # Wb^T: [i, t, o]
    nc.sync.dma_start(WbTS[:, 0:4, :], wbT[:, 0 : 4 * C])
    LD9 = const.tile([RANK, 9, C], bf16)         # lora_down^T: [r, t, i]
    nc.scalar.dma_start(LD9[:], ldT[:])
    LUTn = const.tile([RANK, C], bf16)           # lora_up^T: [r, o]
    nc.scalar.dma_start(LUTn[:], luT[:])
    nc.scalar.dma_start(WbTS[:, 4:6, :], wbT[:, 4 * C : 6 * C])

    # weight tensors the conv loop reads as stationary operands
    wm3 = const.tile([128, 9, C], bf16)          # modulated w^T: [i, t, o]
    R01 = const.tile([128, 3, C], bf16)          # rows ki1+ki2
    R10 = const.tile([128, 3, C], bf16)          # rows ki0+ki1
    comb8 = const.tile([128, 8, C], bf16)        # two-column tap sums (unfolded)
    wm3F = const.tile([128, 9, C], bf16)         # demod-folded variants
    R01F = const.tile([128, 3, C], bf16)
    R10F = const.tile([128, 3, C], bf16)
    comb4F = const.tile([128, 4, C], bf16)       # folded di=1 two-column sums
    demP = const.tile([128, 1], f32)             # rsqrt demod, per output chan
    dmb = const.tile([128, 3], f32)              # de_mod[i], bias[o], 0.25*de_mod
    s2 = const.tile([128, C], f32)               # per-(i,o) tap-summed squares
    onesS = const.tile([128, 1], f32)
    onesRow = const.tile([1, C], f32)

    nc.vector.memset(onesS[:], 1.0)
    nc.vector.memset(onesRow[:], 1.0)

    # ---- input bands: (lo, hi) are halo-inclusive x-row bounds.  x arrives
    # pre-padded on the host ([C, H+2, W+2] with zero borders), so a single
    # whole-tile contiguous DMA delivers data AND halos: no on-device border
    # writes (a disjoint border zero-write races with the DMA's write
    # granules on HW), and per-partition descriptors coalesce.
    segs = [(CUTS[i] - 1, min(CUTS[i + 1], H)) for i in range(len(CUTS) - 1)]
    # DMA issue order: bands 0 and 1 first (consumed earliest), then the
    # LAST band (the tail block is processed early, see below), then the
    # rest in order.
    dma_order = [0, 1, len(segs) - 1] + list(range(2, len(segs) - 1))
    band_tiles = [None] * len(segs)
    for si in dma_order:
        lo, hi = segs[si]
        nrows = hi - lo + 1
        bt = bands.tile([128, nrows, WP], bf16, tag="band", name=f"band{lo}")
        nc.gpsimd.dma_start(bt[:], x[:, lo + 1 : hi + 2, :])
        band_tiles[si] = (bt, lo, hi)
        if si == 0:
            # third WbT chunk rides gpsimd behind the (short) first band
            nc.gpsimd.dma_start(WbTS[:, 6:9, :], wbT[:, 6 * C : 9 * C])

    def _band_for(i0, R):
        for bt, lo, hi in band_tiles:
            if lo <= i0 - 1 and i0 + R <= hi:
                return bt, lo
        raise AssertionError(f"no band for block {i0}")

    # ---- weight stage ----
    with tc.tile_pool(name="wtmp", bufs=1) as wtmp, tc.tile_pool(
        name="wpsum", bufs=1, space="PSUM"
    ) as wpsum:
        dmbP = wpsum.tile([128, 2], f32)
        nc.tensor.transpose(dmbP[:], dmbR[:], id2[:])
        nc.vector.tensor_copy(dmb[:, 0:2], dmbP[:])
        nc.vector.tensor_scalar_mul(dmb[:, 2:3], dmb[:, 0:1], SCALING)

        # deltaT_unscaled[i, t, o] = sum_r down[r,i,t] * up[o,r]; the 0.25
        # lora scale rides in via the fused modulation below
        deltaP = wpsum.tile([128, 9, C], f32)
        for t in range(9):
            nc.tensor.matmul(
                deltaP[:, t, :], LD9[:, t, :], LUTn[:], start=True, stop=True
            )

        # wm3 = Wb^T*dm + deltaT*(0.25*dm), in three tap-slices matching the
        # three WbT DMA chunks so the modulation chain starts on the first
        # bytes to land; the slices the first conv phase reads come first.
        # Row combos over ki (t = 3*ki + kj):
        #   (di=0, a=0): ki0        (di=0, a=1): ki1+ki2
        #   (di=1, a=0): ki0+ki1    (di=1, a=1): ki2
        # Single-column taps are read directly out of wm3/R01/R10; only the
        # two-column sums are materialized (slot = 4*di + 2*a + dj).
        WbTm = wtmp.tile([128, 9, C], bf16)

        def _mod_slice(sl):
            nc.vector.tensor_scalar_mul(WbTm[:, sl, :], WbTS[:, sl, :], dmb[:, 0:1])
            nc.vector.scalar_tensor_tensor(
                wm3[:, sl, :], deltaP[:, sl, :], dmb[:, 2:3], WbTm[:, sl, :],
                op0=ALU.mult, op1=ALU.add,
            )

        _mod_slice(slice(0, 3))
        nc.vector.tensor_add(comb8[:, 0, :], wm3[:, 1, :], wm3[:, 2, :])
        nc.vector.tensor_add(comb8[:, 1, :], wm3[:, 0, :], wm3[:, 1, :])
        _mod_slice(slice(6, 9))
        nc.vector.tensor_add(comb8[:, 6, :], wm3[:, 7, :], wm3[:, 8, :])
        nc.vector.tensor_add(comb8[:, 7, :], wm3[:, 6, :], wm3[:, 7, :])
        _mod_slice(slice(3, 6))
        nc.vector.tensor_add(R01[:], wm3[:, 3:6, :], wm3[:, 6:9, :])
        nc.vector.tensor_add(comb8[:, 2, :], R01[:, 1, :], R01[:, 2, :])
        nc.vector.tensor_add(comb8[:, 3, :], R01[:, 0, :], R01[:, 1, :])
        nc.vector.tensor_add(R10[:], wm3[:, 0:3, :], wm3[:, 3:6, :])
        nc.vector.tensor_add(comb8[:, 4, :], R10[:, 1, :], R10[:, 2, :])
        nc.vector.tensor_add(comb8[:, 5, :], R10[:, 0, :], R10[:, 1, :])
        rowsrc = {
            (0, 0): wm3[:, 0:3, :],
            (0, 1): R01[:],
            (1, 0): R10[:],
            (1, 1): wm3[:, 6:9, :],
        }

        # demod source: sq3 = wm3^2 (ACT), tap-sum on DVE.  The partition
        # sums (PE matmuls) are emitted inside the conv loop so the in-order
        # tensor queue doesn't stall the conv behind this reduce.
        sq3 = wtmp.tile([128, 9, C], f32)
        nc.scalar.square(sq3[:], wm3[:])
        nc.vector.tensor_reduce(
            s2[:], sq3.rearrange("p t o -> p o t"), axis=AX.X, op=ALU.add
        )

    def _conv_weight(di, dj, a, b):
        if dj == 0 and b == 0:
            return rowsrc[(di, a)][:, 0, :]
        if dj == 1 and b == 1:
            return rowsrc[(di, a)][:, 2, :]
        return comb8[:, 4 * di + 2 * a + dj, :]

    rowsrcF = {(1, 0): R10F[:], (1, 1): wm3F[:, 6:9, :]}

    def _conv_weightF(dj, a, b):  # di=1 only
        if dj == 0 and b == 0:
            return rowsrcF[(1, a)][:, 0, :]
        if dj == 1 and b == 1:
            return rowsrcF[(1, a)][:, 2, :]
        return comb4F[:, 2 * a + dj, :]

    def _pair_view(tt, col0):
        """[128, R, W, 2] view of a [128, R, WP] tile: (j, d) -> col j+d+col0."""
        ap = tt[:].copy()
        ap.ap = ap.ap[:-1] + [[1, W], [1, 2]]
        ap.offset = ap.offset + col0
        return ap

    # ---- main conv loop ----
    mpsum = ctx.enter_context(tc.tile_pool(name="mpsum", bufs=7, space="PSUM"))
    spsum = ctx.enter_context(tc.tile_pool(name="spsum", bufs=1, space="PSUM"))
    opool = ctx.enter_context(tc.tile_pool(name="obuf", bufs=3))
    upool = ctx.enter_context(tc.tile_pool(name="ubuf", bufs=3))

    # one PSUM bank shared by the three tiny demod tensors:
    # sP [128,1] | sProw [1,C] at col 4 | demB [128,C] at col 132
    dt_ = spsum.tile([128, 132 + C], f32)

    # Process the short tail block right after the KSWITCH warmup blocks so
    # its eviction+DMA latency hides mid-kernel instead of tailing the run.
    i0s = list(range(0, H, R_BLK))
    i0_order = i0s[:KSWITCH] + [i0s[-1]] + i0s[KSWITCH:-1]
    for bi, i0 in enumerate(i0_order):
        R = min(R_BLK, H - i0)
        bt, lo = _band_for(i0, R)
        # the final block uses the 16-matmul path: its eviction is one
        # engine-level deep, trimming the end-of-kernel latency tail that
        # the 3-op hybrid assembly chain would add
        hybrid = bi >= KSWITCH and R == R_BLK and bi != len(i0_order) - 1
        ob = opool.tile([128, R, 2, 2 * W], bf16, tag="ob", name=f"ob_{i0}")
        obv = ob.rearrange("p r d (j two) -> p r d two j", two=2)

        if not hybrid:
            ph = []
            for p in range(4):
                di, dj = p >> 1, p & 1
                pt = mpsum.tile([128, R * W], f32, tag="ph", name=f"ph{p}_{i0}")
                for q in range(4):
                    a, b = q >> 1, q & 1
                    r0 = i0 + (a + di - 1) - lo      # tile row of first x row
                    co = b + dj - 1
                    rhs = bt[:, r0 : r0 + R, co + 1 : co + 1 + W]
                    nc.tensor.matmul(
                        pt[:], _conv_weight(di, dj, a, b), rhs,
                        start=(q == 0), stop=(q == 3),
                    )
                ph.append(pt)
            if bi == 0:
                # demod partition sums, queued behind block 0's matmuls:
                # sP[o,1] for the eviction scale, sProw[1,o] for the fold.
                sP = dt_[:, 0:1]
                nc.tensor.matmul(sP, s2[:], onesS[:], start=True, stop=True)
                sProw = dt_[0:1, 4 : 4 + C]
                nc.tensor.matmul(sProw, onesS[:], s2[:], start=True, stop=True)
                t1 = const.tile([128, 1], f32)
                nc.vector.tensor_scalar_add(t1[:], sP, EPS)
                t2 = const.tile([128, 1], f32)
                nc.scalar.sqrt(t2[:], t1[:])
                nc.vector.reciprocal(demP[:], t2[:])
                r1 = const.tile([1, C], f32)
                nc.vector.tensor_scalar_add(r1[:], sProw, EPS)
                r2 = const.tile([1, C], f32)
                nc.scalar.sqrt(r2[:], r1[:])
                demRow = const.tile([1, C], f32)
                nc.vector.reciprocal(demRow[:], r2[:])
            if bi == 1:
                # demB[p, o] = dem[o] on every partition (outer product with
                # a ones row); by now demRow is ready so the PE doesn't stall.
                demB = dt_[:, 132 : 132 + C]
                nc.tensor.matmul(demB, onesRow[:], demRow[:], start=True, stop=True)
            # interleave phases into output rows; scale by demod, add bias
            for p in range(4):
                di, dj = p >> 1, p & 1
                dst = obv[:, :, di, dj, :]
                srcv = ph[p].rearrange("p (r j) -> p r j", r=R)
                if dj == 0:
                    nc.vector.tensor_scalar(
                        dst, srcv, demP[:, 0:1], dmb[:, 1:2],
                        op0=ALU.mult, op1=ALU.add,
                    )
                else:
                    nc.scalar.activation(
                        dst, srcv, AF.Identity, bias=dmb[:, 1:2], scale=demP[:, 0:1]
                    )
            if bi == 1:
                # fold demod into the weights for the hybrid blocks
                demB = dt_[:, 132 : 132 + C]
                demBt = demB.unsqueeze(1).broadcast_to([128, 9, C])
                nc.vector.tensor_tensor(wm3F[:], wm3[:], demBt, op=ALU.mult)
                nc.vector.tensor_add(R01F[:], wm3F[:, 3:6, :], wm3F[:, 6:9, :])
                nc.vector.tensor_add(R10F[:], wm3F[:, 0:3, :], wm3F[:, 3:6, :])
                for a in range(2):
                    S = rowsrcF[(1, a)]
                    nc.vector.tensor_add(
                        comb4F[:, 2 * a, :], S[:, 1, :], S[:, 2, :]
                    )
                    nc.vector.tensor_add(
                        comb4F[:, 2 * a + 1, :], S[:, 0, :], S[:, 1, :]
                    )
        else:
            # ---- hybrid 14-matmul block ----
            # di=0: three column-tap tiles t[kj], 2 row-shift matmuls each
            tts = []
            for kj in range(3):
                tt = mpsum.tile([128, R, WP], f32, tag="ph", name=f"tt{kj}_{i0}")
                for a in range(2):
                    wgt = wm3F[:, kj, :] if a == 0 else R01F[:, kj, :]
                    r0 = i0 + a - 1 - lo
                    nc.tensor.matmul(
                        tt[:], wgt, bt[:, r0 : r0 + R, :],
                        start=(a == 0), stop=(a == 1),
                    )
                tts.append(tt)
            # di=1: direct 4-tap phases
            pts = []
            for dj in range(2):
                pt = mpsum.tile([128, R * W], f32, tag="ph", name=f"q{dj}_{i0}")
                for q in range(4):
                    a, b = q >> 1, q & 1
                    r0 = i0 + a - lo
                    co = b + dj - 1
                    rhs = bt[:, r0 : r0 + R, co + 1 : co + 1 + W]
                    nc.tensor.matmul(
                        pt[:], _conv_weightF(dj, a, b), rhs,
                        start=(q == 0), stop=(q == 3),
                    )
                pts.append(pt)
            # di=0 assembly:
            #   y[2i, 2j+d] = t0[j+d] + t1[j+1] + t2[j+1+d] + bias
            # Each op reads at most one PSUM operand (DVE single-PSUM-port):
            #   ACT: sA[j]     = t1[j+1] + bias
            #   DVE: u[j,d]    = t2[j+1+d] + sA[j]     (0-stride pair bcast)
            #   DVE: ob0[j,d]  = t0[j+d] + u[j,d]
            sA = upool.tile([128, R, W], bf16, tag="sA", name=f"sA_{i0}")
            nc.scalar.activation(
                sA[:], tts[1][:, :, 1 : 1 + W], AF.Identity, bias=dmb[:, 1:2]
            )
            u = upool.tile([128, R, 2 * W], bf16, tag="u", name=f"u_{i0}")
            uv = u.rearrange("p r (j d) -> p r j d", d=2)
            sA4 = sA[:].unsqueeze(-1).broadcast_to([128, R, W, 2])
            nc.vector.tensor_tensor(uv, _pair_view(tts[2], 1), sA4, op=ALU.add)
            ob0v = ob[:, :, 0, :].rearrange("p r (j d) -> p r j d", d=2)
            nc.vector.tensor_tensor(ob0v, _pair_view(tts[0], 0), uv, op=ALU.add)
            # di=1 evictions on ACT, bias only (demod already in weights)
            for dj in range(2):
                dst = obv[:, :, 1, dj, :]
                srcv = pts[dj].rearrange("p (r j) -> p r j", r=R)
                nc.scalar.activation(dst, srcv, AF.Identity, bias=dmb[:, 1:2])

        nc.sync.dma_start(y[:, 2 * i0 : 2 * i0 + 2 * R, :], ob[:])


def _build():
    nc = bacc.Bacc(
        "TRN2",
        target_bir_lowering=False,
        debug=False,
        enable_asserts=False,
        num_devices=NCORES,
    )
    x = nc.dram_tensor("x", [C, H + 2, W + 2], bf16, kind="ExternalInput").ap()
    dmbias = nc.dram_tensor("dmbias", [2, C], f32, kind="ExternalInput").ap()
    wbT = nc.dram_tensor("WbT", [C, 9 * C], bf16, kind="ExternalInput").ap()
    luT = nc.dram_tensor("lora_upT", [RANK, C], bf16, kind="ExternalInput").ap()
    ldT = nc.dram_tensor("lora_downT", [RANK, 9 * C], bf16, kind="ExternalInput").ap()
    ident2 = nc.dram_tensor("ident2", [2, 2], f32, kind="ExternalInput").ap()
    y = nc.dram_tensor("y", [C, 2 * H, 2 * W], bf16, kind="ExternalOutput").ap()

    with tile.TileContext(nc) as tc:
        with ExitStack() as ctx:
            _conv_kernel(ctx, tc, y, x, dmbias, wbT, luT, ldT, ident2)
    nc.compile()
    return nc


_CACHE = {}


def _get_nc():
    if "nc" not in _CACHE:
        _CACHE["nc"] = _build()
    return _CACHE["nc"]


def _make_in_maps(x, de_mod, Wb, lora_up, lora_down, bias):
    bf = ml_dtypes.bfloat16
    x = np.asarray(x, dtype=np.float32).astype(bf)
    # zero-pad the spatial borders on the host: the band DMAs then deliver
    # halo rows/columns directly (layout-only prep)
    xp = np.zeros((B, C, H + 2, W + 2), dtype=bf)
    xp[:, :, 1 : H + 1, 1 : W + 1] = x
    de_mod = np.asarray(de_mod, dtype=np.float32)
    Wb = np.asarray(Wb, dtype=np.float32)
    lora_up = np.asarray(lora_up, dtype=np.float32)
    lora_down = np.asarray(lora_down, dtype=np.float32)
    # layout-only host prep: [O,I,3,3] -> [i, (t o)], [R,C,3,3] -> [r, (t i)]
    wbT = np.ascontiguousarray(Wb.transpose(1, 2, 3, 0).reshape(C, 9 * C)).astype(bf)
    luT = np.ascontiguousarray(lora_up.T).astype(bf)
    ldT = np.ascontiguousarray(
        lora_down.transpose(0, 2, 3, 1).reshape(RANK, 9 * C)
    ).astype(bf)
    bias = np.asarray(bias, dtype=np.float32).reshape(C)
    id2 = np.eye(2, dtype=np.float32)
    in_maps = []
    for b in range(NCORES):
        in_maps.append(
            {
                "x": np.ascontiguousarray(xp[b]),
                "dmbias": np.ascontiguousarray(np.stack([de_mod[b], bias])),
                "WbT": wbT,
                "lora_upT": luT,
                "lora_downT": ldT,
                "ident2": id2,
            }
        )
    return in_maps


def run(inputs, trace=False, trace_kwargs=None):
    nc = _get_nc()
    in_maps = _make_in_maps(**inputs)
    res = run_bass_kernel_spmd(
        nc,
        in_maps,
        core_ids=list(range(NCORES)),
        trace=trace,
        **(trace_kwargs or {}),
    )
    y = np.stack(
        [res.results[b]["y"].astype(np.float32) for b in range(NCORES)], axis=0
    )
    return y, res


def kernel(**inputs):
    y, _ = run(inputs)
    return y


# revision 38
# speedup vs baseline: 1.2088x; 1.0074x over previous
"""Trainium2 Bass kernel for nn_NeuronS3DiffUpsample2D.

Reference computation (per sample b):
    up   = nearest-2x-upsample(x[b])                       # [C, 320, 320]
    w    = Wb + 0.25 * einsum('or,rikl->oikl', lora_up, lora_down)
    w_b  = w * de_mod[b, None, :, None, None]              # modulate input chans
    dem  = rsqrt(sum_{i,k,l} w_b^2 + eps)                  # per output chan
    y[b] = conv2d(up, w_b * dem, SAME) + bias

Key algebraic transform: a 3x3 SAME conv on a 2x nearest-upsampled image
decomposes into 4 output phases (di, dj in {0,1}), each a 2x2 conv on the
ORIGINAL 160x160 input:
    y[2i+di, 2j+dj] = sum_{a,b in {0,1}} K[di,dj,a,b] @ x[i+a+di-1, j+b+dj-1]
where the 16 [O, I] matrices K are sums of 1/2/4 of the 9 taps of w.
This is 4/9 of the naive FLOPs and never materializes the upsampled image.

On top of that, the steady-state loop uses a 14-matmul block: the two di=0
phases share three column-tap tiles t[kj][i,j] = sum_a w[S(0,a),kj] (x) at
row shifts (6 matmuls, PE) and are assembled by the DVE as
    y[2i, 2j+dj] = t0[j+dj] + t1[j+1] + t2[j+1+dj] + bias
(shifted overlapping-pair access patterns; t1 uses a 0-stride broadcast
pair).  The di=1 phases stay as direct 4-tap accumulations (8 matmuls)
evicted by the ACT engine.  This trades 2 of 16 PE matmuls per block for
DVE/ACT work that fits in their idle capacity: PE is the only saturated
engine (~90% busy at 1 col/cycle).

For the assembly to be a pure add, the demod scale is folded INTO the
weights.  demod depends on the weights themselves, so the first KSWITCH
blocks run the original 16-matmul path (demod applied per-partition at PSUM
eviction) while the fold chain (row-form demod via a [1,C] PE reduction, a
broadcast outer product, and a DVE multiply) completes in the background.

Sharding: data-parallel over batch B=8 across 8 NeuronCores; each core builds
its own per-sample weights locally.  Host-side work is layout only (slicing,
transposition, fp32->bf16 rounding).  All arithmetic is on device.

Everything runs in bf16 (PE streams bf16 at 1 cycle/row like f32r, but
LDWEIGHTS fully hides under the previous 480-col stream, and input/output
DMA halves; measured rel err 4e-3 vs the 2e-2 budget).  Output goes to DRAM
as bf16 and is widened to f32 on the host.

The input arrives host-padded ([C, 162, 162] with zero borders) so a single
contiguous DMA per band delivers data and halos together — on-device border
zero-writes race with the unaligned bf16 DMA's write granules (observed as
nondeterministic right-edge outputs).  The first band is short so the first
conv matmul isn't gated behind a 1.2MB transfer, and the tail block is
processed early so its eviction+DMA latency hides mid-kernel.  Weight DMAs
are split across the sync and ACT queues; bands ride the gpsimd SWDGE queue.

Measured on 8 axon-tunneled TRN2 cores: 184.1us (vs 225.6us baseline) at
the PE stream roofline (204.5ns per 480-col bf16 matmul, Tensor ~88% busy,
DVE 72%, ACT 68%); run-to-run device clock variance is ~±10%.
"""

import sys
import numpy as np
import ml_dtypes
from contextlib import ExitStack

try:
    import concourse.bass as bass
except ImportError:  # grading env without the axon PYTHONPATH
    sys.path.insert(0, "/opt/trn_rl_repo")
    import concourse.bass as bass
import concourse.tile as tile
from concourse import bacc, mybir
from concourse.bass_utils import run_bass_kernel_spmd

B, C, H, W = 8, 128, 160, 160
RANK = 32
SCALING = 0.25
EPS = 1e-8
WP = W + 2          # padded row length (zero col on each side)
R_BLK = 3           # x-rows per matmul block -> N = 3*160 = 480 <= 512
# band cut points: short first band so block 0 starts early
CUTS = [0, 9, 36, 63, 90, 117, 144, 160]
KSWITCH = 4         # blocks on the 16-matmul path while demod-fold completes
NCORES = 8

f32 = mybir.dt.float32
bf16 = mybir.dt.bfloat16


def _conv_kernel(ctx, tc, y, x, dmbias, wbT, luT, ldT, ident2):
    nc = tc.nc
    AF = mybir.ActivationFunctionType
    ALU = mybir.AluOpType
    AX = mybir.AxisListType

    const = ctx.enter_context(tc.tile_pool(name="const", bufs=1))
    bands = ctx.enter_context(tc.tile_pool(name="bands", bufs=7))

    # dmbias/ident2 (tiny, gate the de_mod transpose) on sync; weight
    # tensors on the otherwise-idle ACT queue; bands on gpsimd SWDGE.
    # The early-queue DMA bandwidth is low (~35GB/s while ramping), so the
    # weight tensors are spread across FOUR queues (sync, vector, ACT;
    # gpsimd carries the bands) to minimize the latest arrival, which gates
    # the whole weight chain and thus the first conv matmul.
    dmbR = const.tile([2, C], f32)
    nc.sync.dma_start(dmbR[:], dmbias[:])
    id2 = const.tile([2, 2], f32)
    nc.sync.dma_start(id2[:], ident2[:])
    WbTS = const.tile([128, 9, C], bf16)         # Wb^T: [i, t, o]
    nc.sync.dma_start(WbTS[:, 0:4, :], wbT[:, 0 : 4 * C])
    LD9 = const.tile([RANK, 9, C], bf16)         # lora_down^T: [r, t, i]
    nc.scalar.dma_start(LD9[:], ldT[:])
    LUTn = const.tile([RANK, C], bf16)           # lora_up^T: [r, o]
    nc.scalar.dma_start(LUTn[:], luT[:])
    nc.scalar.dma_start(WbTS[:, 4:6, :], wbT[:, 4 * C : 6 * C])

    # weight tensors the conv loop reads as stationary operands
    wm3 = const.tile([128, 9, C], bf16)          # modulated w^T: [i, t, o]
    R01 = const.tile([128, 3, C], bf16)          # rows ki1+ki2
    R10 = const.tile([128, 3, C], bf16)          # rows ki0+ki1
    comb8 = const.tile([128, 8, C], bf16)        # two-column tap sums (unfolded)
    wm3F = const.tile([128, 9, C], bf16)         # demod-folded variants
    R01F = const.tile([128, 3, C], bf16)
    R10F = const.tile([128, 3, C], bf16)
    comb4F = const.tile([128, 4, C], bf16)       # folded di=1 two-column sums
    demP = const.tile([128, 1], f32)             # rsqrt demod, per output chan
    dmb = const.tile([128, 3], f32)              # de_mod[i], bias[o], 0.25*de_mod
    s2 = const.tile([128, C], f32)               # per-(i,o) tap-summed squares
    onesS = const.tile([128, 1], f32)
    onesRow = const.tile([1, C], f32)

    nc.vector.memset(onesS[:], 1.0)
    nc.vector.memset(onesRow[:], 1.0)

    # ---- input bands: (lo, hi) are halo-inclusive x-row bounds.  x arrives
    # pre-padded on the host ([C, H+2, W+2] with zero borders), so a single
    # whole-tile contiguous DMA delivers data AND halos: no on-device border
    # writes (a disjoint border zero-write races with the DMA's write
    # granules on HW), and per-partition descriptors coalesce.
    segs = [(CUTS[i] - 1, min(CUTS[i + 1], H)) for i in range(len(CUTS) - 1)]
    # DMA issue order: bands 0 and 1 first (consumed earliest), then the
    # LAST band (the tail block is processed early, see below), then the
    # rest in order.
    dma_order = [0, 1, len(segs) - 1] + list(range(2, len(segs) - 1))
    band_tiles = [None] * len(segs)
    for si in dma_order:
        lo, hi = segs[si]
        nrows = hi - lo + 1
        bt = bands.tile([128, nrows, WP], bf16, tag="band", name=f"band{lo}")
        nc.gpsimd.dma_start(bt[:], x[:, lo + 1 : hi + 2, :])
        band_tiles[si] = (bt, lo, hi)
        if si == 0:
            # third WbT chunk rides gpsimd behind the (short) first band
            nc.gpsimd.dma_start(WbTS[:, 6:9, :], wbT[:, 6 * C : 9 * C])

    def _band_for(i0, R):
        for bt, lo, hi in band_tiles:
            if lo <= i0 - 1 and i0 + R <= hi:
                return bt, lo
        raise AssertionError(f"no band for block {i0}")

    # ---- weight stage ----
    with tc.tile_pool(name="wtmp", bufs=1) as wtmp, tc.tile_pool(
        name="wpsum", bufs=1, space="PSUM"
    ) as wpsum:
        dmbP = wpsum.tile([128, 2], f32)
        nc.tensor.transpose(dmbP[:], dmbR[:], id2[:])
        nc.vector.tensor_copy(dmb[:, 0:2], dmbP[:])
        nc.vector.tensor_scalar_mul(dmb[:, 2:3], dmb[:, 0:1], SCALING)

        # deltaT_unscaled[i, t, o] = sum_r down[r,i,t] * up[o,r]; the 0.25
        # lora scale rides in via the fused modulation below
        deltaP = wpsum.tile([128, 9, C], f32)
        for t in range(9):
            nc.tensor.matmul(
                deltaP[:, t, :], LD9[:, t, :], LUTn[:], start=True, stop=True
            )

        # wm3 = Wb^T*dm + deltaT*(0.25*dm), in three tap-slices matching the
        # three WbT DMA chunks so the modulation chain starts on the first
        # bytes to land; the slices the first conv phase reads come first.
        # Row combos over ki (t = 3*ki + kj):
        #   (di=0, a=0): ki0        (di=0, a=1): ki1+ki2
        #   (di=1, a=0): ki0+ki1    (di=1, a=1): ki2
        # Single-column taps are read directly out of wm3/R01/R10; only the
        # two-column sums are materialized (slot = 4*di + 2*a + dj).
        WbTm = wtmp.tile([128, 9, C], bf16)

        def _mod_slice(sl):
            nc.vector.tensor_scalar_mul(WbTm[:, sl, :], WbTS[:, sl, :], dmb[:, 0:1])
            nc.vector.scalar_tensor_tensor(
                wm3[:, sl, :], deltaP[:, sl, :], dmb[:, 2:3], WbTm[:, sl, :],
                op0=ALU.mult, op1=ALU.add,
            )

        # all three wm3 slices first (the conv matmuls' dependency on wm3
        # resolves at its LAST write, so slot builds must not interleave),
        # ordered by expected chunk arrival: sync(0:4), scalar(4:6),
        # gpsimd(6:9 — behind band0, lands last)
        _mod_slice(slice(0, 3))
        _mod_slice(slice(3, 6))
        _mod_slice(slice(6, 9))
        nc.vector.tensor_add(comb8[:, 0, :], wm3[:, 1, :], wm3[:, 2, :])
        nc.vector.tensor_add(comb8[:, 1, :], wm3[:, 0, :], wm3[:, 1, :])
        nc.vector.tensor_add(comb8[:, 6, :], wm3[:, 7, :], wm3[:, 8, :])
        nc.vector.tensor_add(comb8[:, 7, :], wm3[:, 6, :], wm3[:, 7, :])
        nc.vector.tensor_add(R01[:], wm3[:, 3:6, :], wm3[:, 6:9, :])
        nc.vector.tensor_add(comb8[:, 2, :], R01[:, 1, :], R01[:, 2, :])
        nc.vector.tensor_add(comb8[:, 3, :], R01[:, 0, :], R01[:, 1, :])
        nc.vector.tensor_add(R10[:], wm3[:, 0:3, :], wm3[:, 3:6, :])
        nc.vector.tensor_add(comb8[:, 4, :], R10[:, 1, :], R10[:, 2, :])
        nc.vector.tensor_add(comb8[:, 5, :], R10[:, 0, :], R10[:, 1, :])
        rowsrc = {
            (0, 0): wm3[:, 0:3, :],
            (0, 1): R01[:],
            (1, 0): R10[:],
            (1, 1): wm3[:, 6:9, :],
        }

        # demod source: sq3 = wm3^2 (ACT), tap-sum on DVE.  The partition
        # sums (PE matmuls) are emitted inside the conv loop so the in-order
        # tensor queue doesn't stall the conv behind this reduce.
        sq3 = wtmp.tile([128, 9, C], f32)
        nc.scalar.square(sq3[:], wm3[:])
        nc.vector.tensor_reduce(
            s2[:], sq3.rearrange("p t o -> p o t"), axis=AX.X, op=ALU.add
        )

    def _conv_weight(di, dj, a, b):
        if dj == 0 and b == 0:
            return rowsrc[(di, a)][:, 0, :]
        if dj == 1 and b == 1:
            return rowsrc[(di, a)][:, 2, :]
        return comb8[:, 4 * di + 2 * a + dj, :]

    rowsrcF = {(1, 0): R10F[:], (1, 1): wm3F[:, 6:9, :]}

    def _conv_weightF(dj, a, b):  # di=1 only
        if dj == 0 and b == 0:
            return rowsrcF[(1, a)][:, 0, :]
        if dj == 1 and b == 1:
            return rowsrcF[(1, a)][:, 2, :]
        return comb4F[:, 2 * a + dj, :]

    def _pair_view(tt, col0):
        """[128, R, W, 2] view of a [128, R, WP] tile: (j, d) -> col j+d+col0."""
        ap = tt[:].copy()
        ap.ap = ap.ap[:-1] + [[1, W], [1, 2]]
        ap.offset = ap.offset + col0
        return ap

    # ---- main conv loop ----
    mpsum = ctx.enter_context(tc.tile_pool(name="mpsum", bufs=7, space="PSUM"))
    spsum = ctx.enter_context(tc.tile_pool(name="spsum", bufs=1, space="PSUM"))
    opool = ctx.enter_context(tc.tile_pool(name="obuf", bufs=3))
    upool = ctx.enter_context(tc.tile_pool(name="ubuf", bufs=3))

    # one PSUM bank shared by the three tiny demod tensors:
    # sP [128,1] | sProw [1,C] at col 4 | demB [128,C] at col 132
    dt_ = spsum.tile([128, 132 + C], f32)

    # Process the short tail block right after the KSWITCH warmup blocks so
    # its eviction+DMA latency hides mid-kernel instead of tailing the run.
    i0s = list(range(0, H, R_BLK))
    i0_order = i0s[:KSWITCH] + [i0s[-1]] + i0s[KSWITCH:-1]
    for bi, i0 in enumerate(i0_order):
        R = min(R_BLK, H - i0)
        bt, lo = _band_for(i0, R)
        # the final block uses the 16-matmul path: its eviction is one
        # engine-level deep, trimming the end-of-kernel latency tail that
        # the 3-op hybrid assembly chain would add
        hybrid = bi >= KSWITCH and R == R_BLK and bi != len(i0_order) - 1
        ob = opool.tile([128, R, 2, 2 * W], bf16, tag="ob", name=f"ob_{i0}")
        obv = ob.rearrange("p r d (j two) -> p r d two j", two=2)

        if not hybrid:
            ph = []
            for p in range(4):
                di, dj = p >> 1, p & 1
                pt = mpsum.tile([128, R * W], f32, tag="ph", name=f"ph{p}_{i0}")
                for q in range(4):
                    a, b = q >> 1, q & 1
                    r0 = i0 + (a + di - 1) - lo      # tile row of first x row
                    co = b + dj - 1
                    rhs = bt[:, r0 : r0 + R, co + 1 : co + 1 + W]
                    nc.tensor.matmul(
                        pt[:], _conv_weight(di, dj, a, b), rhs,
                        start=(q == 0), stop=(q == 3),
                    )
                ph.append(pt)
            if bi == 0:
                # demod partition sums, queued behind block 0's matmuls:
                # sP[o,1] for the eviction scale, sProw[1,o] for the fold.
                sP = dt_[:, 0:1]
                nc.tensor.matmul(sP, s2[:], onesS[:], start=True, stop=True)
                sProw = dt_[0:1, 4 : 4 + C]
                nc.tensor.matmul(sProw, onesS[:], s2[:], start=True, stop=True)
                t1 = const.tile([128, 1], f32)
                nc.vector.tensor_scalar_add(t1[:], sP, EPS)
                t2 = const.tile([128, 1], f32)
                nc.scalar.sqrt(t2[:], t1[:])
                nc.vector.reciprocal(demP[:], t2[:])
                r1 = const.tile([1, C], f32)
                nc.vector.tensor_scalar_add(r1[:], sProw, EPS)
                r2 = const.tile([1, C], f32)
                nc.scalar.sqrt(r2[:], r1[:])
                demRow = const.tile([1, C], f32)
                nc.vector.reciprocal(demRow[:], r2[:])
            if bi == 1:
                # demB[p, o] = dem[o] on every partition (outer product with
                # a ones row); by now demRow is ready so the PE doesn't stall.
                demB = dt_[:, 132 : 132 + C]
                nc.tensor.matmul(demB, onesRow[:], demRow[:], start=True, stop=True)
            # interleave phases into output rows; scale by demod, add bias
            for p in range(4):
                di, dj = p >> 1, p & 1
                dst = obv[:, :, di, dj, :]
                srcv = ph[p].rearrange("p (r j) -> p r j", r=R)
                if dj == 0:
                    nc.vector.tensor_scalar(
                        dst, srcv, demP[:, 0:1], dmb[:, 1:2],
                        op0=ALU.mult, op1=ALU.add,
                    )
                else:
                    nc.scalar.activation(
                        dst, srcv, AF.Identity, bias=dmb[:, 1:2], scale=demP[:, 0:1]
                    )
            if bi == 1:
                # fold demod into the weights for the hybrid blocks
                demB = dt_[:, 132 : 132 + C]
                demBt = demB.unsqueeze(1).broadcast_to([128, 9, C])
                nc.vector.tensor_tensor(wm3F[:], wm3[:], demBt, op=ALU.mult)
                nc.vector.tensor_add(R01F[:], wm3F[:, 3:6, :], wm3F[:, 6:9, :])
                nc.vector.tensor_add(R10F[:], wm3F[:, 0:3, :], wm3F[:, 3:6, :])
                for a in range(2):
                    S = rowsrcF[(1, a)]
                    nc.vector.tensor_add(
                        comb4F[:, 2 * a, :], S[:, 1, :], S[:, 2, :]
                    )
                    nc.vector.tensor_add(
                        comb4F[:, 2 * a + 1, :], S[:, 0, :], S[:, 1, :]
                    )
        else:
            # ---- hybrid 14-matmul block ----
            # di=0: three column-tap tiles t[kj], 2 row-shift matmuls each
            tts = []
            for kj in range(3):
                tt = mpsum.tile([128, R, WP], f32, tag="ph", name=f"tt{kj}_{i0}")
                for a in range(2):
                    wgt = wm3F[:, kj, :] if a == 0 else R01F[:, kj, :]
                    r0 = i0 + a - 1 - lo
                    nc.tensor.matmul(
                        tt[:], wgt, bt[:, r0 : r0 + R, :],
                        start=(a == 0), stop=(a == 1),
                    )
                tts.append(tt)
            # di=1: direct 4-tap phases
            pts = []
            for dj in range(2):
                pt = mpsum.tile([128, R * W], f32, tag="ph", name=f"q{dj}_{i0}")
                for q in range(4):
                    a, b = q >> 1, q & 1
                    r0 = i0 + a - lo
                    co = b + dj - 1
                    rhs = bt[:, r0 : r0 + R, co + 1 : co + 1 + W]
                    nc.tensor.matmul(
                        pt[:], _conv_weightF(dj, a, b), rhs,
                        start=(q == 0), stop=(q == 3),
                    )
                pts.append(pt)
            # di=0 assembly:
            #   y[2i, 2j+d] = t0[j+d] + t1[j+1] + t2[j+1+d] + bias
            # Each op reads at most one PSUM operand (DVE single-PSUM-port):
            #   ACT: sA[j]     = t1[j+1] + bias
            #   DVE: u[j,d]    = t2[j+1+d] + sA[j]     (0-stride pair bcast)
            #   DVE: ob0[j,d]  = t0[j+d] + u[j,d]
            sA = upool.tile([128, R, W], bf16, tag="sA", name=f"sA_{i0}")
            nc.scalar.activation(
                sA[:], tts[1][:, :, 1 : 1 + W], AF.Identity, bias=dmb[:, 1:2]
            )
            u = upool.tile([128, R, 2 * W], bf16, tag="u", name=f"u_{i0}")
            uv = u.rearrange("p r (j d) -> p r j d", d=2)
            sA4 = sA[:].unsqueeze(-1).broadcast_to([128, R, W, 2])
            nc.vector.tensor_tensor(uv, _pair_view(tts[2], 1), sA4, op=ALU.add)
            ob0v = ob[:, :, 0, :].rearrange("p r (j d) -> p r j d", d=2)
            nc.vector.tensor_tensor(ob0v, _pair_view(tts[0], 0), uv, op=ALU.add)
            # di=1 evictions on ACT, bias only (demod already in weights)
            for dj in range(2):
                dst = obv[:, :, 1, dj, :]
                srcv = pts[dj].rearrange("p (r j) -> p r j", r=R)
                nc.scalar.activation(dst, srcv, AF.Identity, bias=dmb[:, 1:2])

        nc.sync.dma_start(y[:, 2 * i0 : 2 * i0 + 2 * R, :], ob[:])


def _build():
    nc = bacc.Bacc(
        "TRN2",
        target_bir_lowering=False,
        debug=False,
        enable_asserts=False,
        num_devices=NCORES,
    )
    x = nc.dram_tensor("x", [C, H + 2, W + 2], bf16, kind="ExternalInput").ap()
    dmbias = nc.dram_tensor("dmbias", [2, C], f32, kind="ExternalInput").ap()
    wbT = nc.dram_tensor("WbT", [C, 9 * C], bf16, kind="ExternalInput").ap()
    luT = nc.dram_tensor("lora_upT", [RANK, C], bf16, kind="ExternalInput").ap()
    ldT = nc.dram_tensor("lora_downT", [RANK, 9 * C], bf16, kind="ExternalInput").ap()
    ident2 = nc.dram_tensor("ident2", [2, 2], f32, kind="ExternalInput").ap()
    y = nc.dram_tensor("y", [C, 2 * H, 2 * W], bf16, kind="ExternalOutput").ap()

    with tile.TileContext(nc) as tc:
        with ExitStack() as ctx:
            _conv_kernel(ctx, tc, y, x, dmbias, wbT, luT, ldT, ident2)
    nc.compile()
    return nc


_CACHE = {}


def _get_nc():
    if "nc" not in _CACHE:
        _CACHE["nc"] = _build()
    return _CACHE["nc"]


def _make_in_maps(x, de_mod, Wb, lora_up, lora_down, bias):
    bf = ml_dtypes.bfloat16
    x = np.asarray(x, dtype=np.float32).astype(bf)
    # zero-pad the spatial borders on the host: the band DMAs then deliver
    # halo rows/columns directly (layout-only prep)
    xp = np.zeros((B, C, H + 2, W + 2), dtype=bf)
    xp[:, :, 1 : H + 1, 1 : W + 1] = x
    de_mod = np.asarray(de_mod, dtype=np.float32)
    Wb = np.asarray(Wb, dtype=np.float32)
    lora_up = np.asarray(lora_up, dtype=np.float32)
    lora_down = np.asarray(lora_down, dtype=np.float32)
    # layout-only host prep: [O,I,3,3] -> [i, (t o)], [R,C,3,3] -> [r, (t i)]
    wbT = np.ascontiguousarray(Wb.transpose(1, 2, 3, 0).reshape(C, 9 * C)).astype(bf)
    luT = np.ascontiguousarray(lora_up.T).astype(bf)
    ldT = np.ascontiguousarray(
        lora_down.transpose(0, 2, 3, 1).reshape(RANK, 9 * C)
    ).astype(bf)
    bias = np.asarray(bias, dtype=np.float32).reshape(C)
    id2 = np.eye(2, dtype=np.float32)
    in_maps = []
    for b in range(NCORES):
        in_maps.append(
            {
                "x": np.ascontiguousarray(xp[b]),
                "dmbias": np.ascontiguousarray(np.stack([de_mod[b], bias])),
                "WbT": wbT,
                "lora_upT": luT,
                "lora_downT": ldT,
                "ident2": id2,
            }
        )
    return in_maps


def run(inputs, trace=False, trace_kwargs=None):
    nc = _get_nc()
    in_maps = _make_in_maps(**inputs)
    res = run_bass_kernel_spmd(
        nc,
        in_maps,
        core_ids=list(range(NCORES)),
        trace=trace,
        **(trace_kwargs or {}),
    )
    y = np.stack(
        [res.results[b]["y"].astype(np.float32) for b in range(NCORES)], axis=0
    )
    return y, res


def kernel(**inputs):
    y, _ = run(inputs)
    return y
